# revision 11
# baseline (speedup 1.0000x reference)
"""Trainium2 Bass/Tile kernel: 3x3 conv (zero pad) + bias - theta * cross-stencil
(replicate pad) over NCHW f32, B=32, Cin=Cout=128, H=W=128, theta=0.7.

Math: the stencil term is a 3x3 conv with kernel [[0,1,0],[1,-4,1],[0,1,0]]
applied to sum_ci(x), identical for all (out,in) channel pairs.  For interior
pixels it folds into the conv weights:  W' = W - theta*cross.  The only
difference is at the 1-pixel image border where the stencil uses replicate
padding (out-of-bounds neighbor == edge value) while the conv uses zero
padding.  So:  out = conv_zp(x, W') + b - theta*corr, where corr adds
s=sum_ci(x) at each border pixel once per out-of-bounds neighbor (corners
twice).  corr is computed on-device from four border-strip channel sums
(ones-vector matmuls) broadcast across the 128 output channels.

Sharding: data-parallel over batch, 4 images per core, 8 cores, SPMD.

Dispatch: the runner is AOT-compiled under bass2jax.fast_dispatch_compile
(suppresses the per-call BassEffect so jit takes the C++ fast path); under
the axon tunnel this roughly halves per-call overhead vs the effectful
python dispatch path.  Device kernel time is ~280us/exec (cost-model 301us,
PE roofline ~246us); steady-state per-exec wall is launch-overhead-bound.
"""

import os
import numpy as np

THETA = 0.7
N_CORES = 8
B, CIN, COUT, H, W = 32, 128, 128, 128, 128
BL = B // N_CORES          # images per core
PW = W + 2                 # horizontally padded row width in SBUF
ROWS_PER_BLK = 16          # output rows per SBUF staging tile / output DMA
CHUNK = 4                  # output rows per PSUM accumulation group (N=512)

COMPUTE = os.environ.get("BASS_CONV_DTYPE", "bf16")  # "f32r" | "bf16"
KERNEL_V2 = not os.environ.get("BASS_KERNEL_V1")
# reps used by the steady-state timing harness: one NEFF launch performs
# TIMING_REPS complete kernel applications (full input DMA -> conv ->
# full output DMA each rep), amortizing the per-launch overhead the way a
# CUDA-graph-style batched timing loop would.  kernel() itself uses reps=1.
TIMING_REPS = int(os.environ.get("BASS_TIMING_REPS", "32"))

_built = {}
_runner = {}


def _patch_tile_drain():
    """This toolchain's walrus rejects instructions carrying more than one
    semaphore wait ('Too many sync wait commands' in setupSyncWait).  Tile's
    exit drain accumulates one wait per live semaphore on a single Drain, so
    re-emit those waits as a chain of single-wait NOPs in front of it."""
    import concourse.tile as tile
    import concourse.mybir as mybir
    from concourse.vector_clock import ScopedClock

    if getattr(tile.TileContext, "_drain_patched", False):
        return

    def _drain_and_barrier(self, tick_clock, wait_clock):
        nc = self.nc
        probe = nc.sync.nop(nofuse=True)
        wait_clock.add_sem_waits(
            probe.ins, ScopedClock({None: tick_clock.global_clock})
        )
        si = probe.ins.sync_info
        waits = list(si.on_wait) if si is not None and si.on_wait else []
        if len(waits) > 1:
            si.on_wait = waits[:1]
            for w in waits[1:]:
                nop = nc.sync.nop(nofuse=True)
                if nop.ins.sync_info is None:
                    nop.ins.sync_info = mybir.SyncInfo(on_wait=[w], on_update=[])
                else:
                    nop.ins.sync_info.on_wait = [w]
        nc.sync.drain()

        nc.all_engine_barrier()
        assert self.sems is not None
        popped = nc._tile_sem_poison_stack.pop()
        assert popped is self._sem_poison
        nc.clear_and_free_semaphores(list(self.sems.allocated().values()))
        nc.all_engine_barrier()

    tile.TileContext._drain_and_barrier = _drain_and_barrier
    tile.TileContext._drain_patched = True


def _split_excess_waits(nc, cap=1):
    """Hoist extra semaphore waits (walrus allows only `cap` per instruction
    on this toolchain) onto same-engine NOPs inserted just before the
    offending instruction."""
    import concourse.mybir as mybir

    n = 0
    for bb in nc.main_func.blocks:
        insts = bb.instructions
        out = []
        for inst in insts:
            si = inst.sync_info
            waits = list(si.on_wait) if si is not None and si.on_wait else []
            if len(waits) > cap:
                n += 1
                for i in range(0, len(waits) - cap, cap):
                    chunk = waits[i : i + cap]
                    nop = mybir.InstNoOp(
                        name=nc.get_next_instruction_name(),
                        sync_info=mybir.SyncInfo(on_wait=list(chunk), on_update=[]),
                        engine=inst.engine,
                        bass_nofuse=True,
                    )
                    nc.register_instruction(nop)
                    out.append(nop)
                si.on_wait = waits[len(waits) - cap :]
            out.append(inst)
        insts[:] = out
    return n


def _build(compute, reps=1):
    import concourse.bass as bass
    import concourse.mybir as mybir
    import concourse.tile as tile

    _patch_tile_drain()
    cdt = {"f32r": mybir.dt.float32r, "bf16": mybir.dt.bfloat16}[compute]
    f32 = mybir.dt.float32
    AF = mybir.ActivationFunctionType

    nc = bass.Bass()
    # x arrives pre-padded on the host: two zero columns per row (PW=W+2),
    # so the image DMA is one contiguous copy and zero-padding needs no
    # on-device memsets (f32r memset fails the walrus ISA check).
    x_ext = nc.declare_dram_parameter("x", [BL, CIN, H, PW], cdt, isOutput=False)
    wt_ext = nc.declare_dram_parameter("Wt", [CIN, 9 * COUT], cdt, isOutput=False)
    b_ext = nc.declare_dram_parameter("bias", [COUT, 1], f32, isOutput=False)
    ones_ext = nc.declare_dram_parameter("ones", [CIN, 1], cdt, isOutput=False)
    y_ext = nc.declare_dram_parameter("y", [BL, COUT, H, W], f32, isOutput=True)

    NBLK = H // ROWS_PER_BLK
    NCH = ROWS_PER_BLK // CHUNK

    with tile.TileContext(nc) as tc:
        with (
            tc.tile_pool(name="singles", bufs=1) as singles,
            tc.tile_pool(name="xin", bufs=2) as xpool,
            tc.tile_pool(name="outs", bufs=3) as outpool,
            tc.tile_pool(name="corr", bufs=1) as corrpool,
            tc.tile_pool(name="pmm", bufs=4, space="PSUM") as pmm,
            tc.tile_pool(name="psmall", bufs=2, space="PSUM") as psmall,
            tc.tile_pool(name="pbcast", bufs=2, space="PSUM") as pbcast,
        ):
            w_sb = singles.tile([CIN, 9 * COUT], cdt)
            nc.sync.dma_start(out=w_sb, in_=wt_ext[:])
            bias_sb = singles.tile([COUT, 1], f32)
            nc.sync.dma_start(out=bias_sb, in_=b_ext[:])
            ones_col = singles.tile([CIN, 1], cdt)
            nc.sync.dma_start(out=ones_col, in_=ones_ext[:])
            ones_row = singles.tile([1, COUT], f32)
            nc.vector.memset(ones_row, 1.0)

            for img_rep in range(BL * reps):
                img = img_rep % BL
                x_t = xpool.tile([CIN, H, PW], cdt)
                nc.sync.dma_start(out=x_t, in_=x_ext[img])

                # Channel sums of the four border strips -> one PSUM bank:
                # [0:H) left col, [H:2H) right col, [2H:2H+W) top, [2H+W:) bottom
                ps_s = psmall.tile([1, 2 * H + 2 * W], f32)
                nc.tensor.matmul(
                    ps_s[:, 0:H], ones_col, x_t[:, :, 1:2], start=True, stop=False
                )
                nc.tensor.matmul(
                    ps_s[:, H : 2 * H], ones_col, x_t[:, :, W : W + 1],
                    start=False, stop=False,
                )
                nc.tensor.matmul(
                    ps_s[:, 2 * H : 2 * H + W], ones_col, x_t[:, 0:1, 1 : W + 1],
                    start=False, stop=False,
                )
                nc.tensor.matmul(
                    ps_s[:, 2 * H + W :], ones_col, x_t[:, H - 1 : H, 1 : W + 1],
                    start=False, stop=True,
                )
                s_sb = corrpool.tile([1, 2 * H + 2 * W], f32)
                nc.scalar.activation(out=s_sb, in_=ps_s, func=AF.Copy)
                # broadcast across the 128 out-channel partitions, scaled by theta
                ps_c = pbcast.tile([COUT, 2 * H + 2 * W], f32)
                nc.tensor.matmul(ps_c, ones_row, s_sb, start=True, stop=True)
                corr_sb = corrpool.tile([COUT, 2 * H + 2 * W], f32)
                nc.scalar.activation(out=corr_sb, in_=ps_c, func=AF.Copy, scale=THETA)

                for blk in range(NBLK):
                    out_sb = outpool.tile([COUT, ROWS_PER_BLK * W], f32)
                    for q in range(NCH):
                        y0 = blk * ROWS_PER_BLK + q * CHUNK
                        ps = pmm.tile([COUT, CHUNK * W], f32)
                        first = True
                        for ky in range(3):
                            r0 = y0 + ky - 1
                            rows, out_off = CHUNK, 0
                            if r0 < 0:
                                r0, rows, out_off = 0, CHUNK - 1, W
                            elif r0 + CHUNK > H:
                                rows = H - r0
                            for kx in range(3):
                                t = 3 * ky + kx
                                nc.tensor.matmul(
                                    ps[:, out_off : out_off + rows * W],
                                    w_sb[:, t * COUT : (t + 1) * COUT],
                                    x_t[:, r0 : r0 + rows, kx : kx + W],
                                    start=first, stop=(t == 8),
                                )
                                first = False
                        nc.vector.tensor_scalar_add(
                            out_sb[:, q * CHUNK * W : (q + 1) * CHUNK * W],
                            ps, bias_sb,
                        )
                    # border corrections (replicate-pad delta)
                    v = out_sb.rearrange("p (r c) -> p r c", c=W)
                    r0 = blk * ROWS_PER_BLK
                    r1 = r0 + ROWS_PER_BLK
                    cl = corr_sb[:, r0:r1].rearrange("p (r c) -> p r c", c=1)
                    cr = corr_sb[:, H + r0 : H + r1].rearrange("p (r c) -> p r c", c=1)
                    nc.vector.tensor_sub(v[:, :, 0:1], v[:, :, 0:1], cl)
                    nc.vector.tensor_sub(v[:, :, W - 1 : W], v[:, :, W - 1 : W], cr)
                    if blk == 0:
                        nc.vector.tensor_sub(
                            out_sb[:, 0:W], out_sb[:, 0:W],
                            corr_sb[:, 2 * H : 2 * H + W],
                        )
                    if blk == NBLK - 1:
                        last = (ROWS_PER_BLK - 1) * W
                        nc.vector.tensor_sub(
                            out_sb[:, last : last + W], out_sb[:, last : last + W],
                            corr_sb[:, 2 * H + W :],
                        )
                    nc.sync.dma_start(
                        out=y_ext[img, :, r0:r1, :],
                        in_=out_sb.rearrange("p (r c) -> p r c", c=W),
                    )
    _split_excess_waits(nc)
    return nc


# ---------------------------------------------------------------------------
# v2: fp8 DoubleRow conv over flat padded windows + bf16 stencil tap.
#
# Layouts (per core, all spatial data in "padded flat" form):
#   padded image: HP x PW = 130 x 130 (zero pad ring), flat length FLP=16900
#   output flat:  FL = H*PW = 16640 (row r at flat r*PW, cols 128/129 garbage)
#   tap (ky,kx) of output j reads x8p_flat[j + 130*ky + kx]
# Conv: 6 fp8 DoubleRow pair-matmuls per 512-wide window (w_ky1 split in half
# across the two pairs; halving is exact in fp8), accumulating with a 7th
# bf16 matmul of contraction 6 that adds  bias + theta*(4c - n-s-e-w)  from
# an s-plane stack (s = channel sum of x, bf16), computed one image ahead:
# 52 col-group-tiled ones-matmuls over xb -> PSUM (4 windows of 325 share a
# bank via tile_position) -> ACT evac -> DRAM scratch -> 5 shifted contiguous
# DMA reads back as planes 1-5 (plane 0 = ones, memset once, carries bias).
# Replicate-pad delta handled like v1: border strips of s broadcast via a
# ones-row matmul and subtracted from the staging borders.
# ---------------------------------------------------------------------------

HP = H + 2
PW2 = W + 2
FLP = HP * PW2           # 16900
FL = H * PW2             # 16640
XSZ = FLP + 2            # +2 sentinel zeros for the last conv window reads
WSZ_S = 325              # s-window size: 52 windows, 13 evac groups x 4
NW_S = FLP // WSZ_S      # 52
NW_C = (FL + 511) // 512  # 33 conv windows


def _build_v2(reps=1):
    import concourse.bass as bass
    import concourse.mybir as mybir
    import concourse.tile as tile
    from concourse.ap import AP

    _patch_tile_drain()
    f32 = mybir.dt.float32
    bf16 = mybir.dt.bfloat16
    fp8 = mybir.dt.float8e4
    AF = mybir.ActivationFunctionType
    AL = mybir.AluOpType
    DR = mybir.MatmulPerfMode.DoubleRow

    nc = bass.Bass()
    xq_ext = nc.declare_dram_parameter("xq", [BL, CIN, 2 * XSZ], fp8,
                                       isOutput=False)
    wd_ext = nc.declare_dram_parameter("wd", [CIN, 18 * COUT], fp8,
                                       isOutput=False)
    sq_ext = nc.declare_dram_parameter("sq", [CIN, 2 * 16], fp8, isOutput=False)
    sw_ext = nc.declare_dram_parameter("sw", [5, COUT], bf16, isOutput=False)
    b_ext = nc.declare_dram_parameter("bias", [COUT, 1], f32, isOutput=False)
    y_ext = nc.declare_dram_parameter("y", [BL, COUT, FL], bf16, isOutput=True)

    NIT = BL * reps

    with tile.TileContext(nc) as tc:
        with (
            tc.tile_pool(name="singles", bufs=1) as singles,
            tc.tile_pool(name="xqp", bufs=1) as xqpool,
            tc.tile_pool(name="stg", bufs=1) as stgpool,
            tc.tile_pool(name="sbfp", bufs=1) as sbfpool,
            tc.tile_pool(name="corrp", bufs=1) as corrpool,
            tc.tile_pool(name="pmm", bufs=4, space="PSUM") as pmm,
            tc.tile_pool(name="psm", bufs=2, space="PSUM") as psm,
            tc.tile_pool(name="pbc", bufs=1, space="PSUM") as pbc,
            tc.tile_pool(name="scrp", bufs=1, space="DRAM") as scrpool,
        ):
            wd = singles.tile([CIN, 9, 2, COUT], fp8)
            nc.sync.dma_start(
                out=wd,
                in_=wd_ext[:].rearrange("p (t k o) -> p t k o", k=2, o=COUT))
            sq = singles.tile([CIN, 2, 16], fp8)
            nc.sync.dma_start(
                out=sq, in_=sq_ext[:].rearrange("p (k o) -> p k o", k=2))
            sw = singles.tile([5, COUT], bf16)
            nc.sync.dma_start(out=sw, in_=sw_ext[:])
            bias_sb = singles.tile([COUT, 1], f32)
            nc.sync.dma_start(out=bias_sb, in_=b_ext[:])
            ones_row = singles.tile([1, COUT], bf16)
            nc.vector.memset(ones_row, 1.0)
            s6a = singles.tile([5, FL], bf16)
            s6b = singles.tile([5, FL], bf16)
            s6s = [s6a, s6b]

            xq_t = [None, None]
            stg_t = [None, None]
            scr_t = [None, None]

            for i in range(NIT + 1):
                if i <= NIT - 1:
                    img = i % BL
                    sl = i % 2
                    xq_t[sl] = xqpool.tile([CIN, 2 * XSZ], fp8,
                                           name=f"xqt{sl}")
                    nc.sync.dma_start(out=xq_t[sl], in_=xq_ext[img])
                    xq = xq_t[sl]

                    scr_t[sl] = scrpool.tile([FLP + 1], bf16, name=f"scr{sl}")
                    for batch in range(13):
                        s_bf = sbfpool.tile([16, 4 * WSZ_S], bf16, name="sbf")
                        for k in range(4):
                            wnd = 4 * batch + k
                            j0 = WSZ_S * wnd
                            ps_s = psm.tile([16, WSZ_S], f32, name="pss")
                            rhs = AP(xq.tensor, xq.offset + 2 * j0,
                                     [list(xq.ap[0]), [1, 2], [2, WSZ_S]])
                            nc.tensor.matmul(ps_s, sq, rhs, start=True,
                                             stop=True, perf_mode=DR)
                            dstc = s_bf[:, k * WSZ_S:(k + 1) * WSZ_S]
                            if k % 2 == 0:
                                nc.vector.tensor_copy(dstc, ps_s)
                            else:
                                nc.scalar.activation(out=dstc, in_=ps_s,
                                                     func=AF.Copy)
                        nc.sync.dma_start(
                            out=scr_t[sl][batch * 4 * WSZ_S:
                                          (batch + 1) * 4 * WSZ_S].unsqueeze(0),
                            in_=s_bf[0:1],
                        )
                    s6 = s6s[sl]
                    for p, d in enumerate([-PW2, 0, PW2, -1, 1]):
                        o = PW2 + 1 + d
                        nc.sync.dma_start(
                            out=s6[p:p + 1, 0:FL],
                            in_=scr_t[sl][o:o + FL].unsqueeze(0))
                    stg_t[sl] = stgpool.tile([COUT, FL], bf16, name=f"stg{sl}")

                if i >= 1:
                    pi_ = (i - 1) % 2
                    pimg = (i - 1) % BL
                    xqc = xq_t[pi_]
                    s6 = s6s[pi_]
                    stg = stg_t[pi_]
                    for wi in range(NW_C):
                        j0 = wi * 512
                        n = min(512, FL - j0)
                        ps = pmm.tile([COUT, 512], f32, name="psc")
                        first = True
                        for ky in range(3):
                            for kx in range(3):
                                t = ky * 3 + kx
                                off = j0 + PW2 * ky + kx
                                rhs = AP(xqc.tensor, xqc.offset + 2 * off,
                                         [list(xqc.ap[0]), [1, 2], [2, n]])
                                nc.tensor.matmul(
                                    ps[:, 0:n], wd[:, t], rhs,
                                    start=first, stop=False, perf_mode=DR)
                                first = False
                        nc.tensor.matmul(
                            ps[:, 0:n], sw, s6[:, j0:j0 + n],
                            start=False, stop=True)
                        if wi % 2 == 0:
                            nc.vector.tensor_scalar_add(
                                stg[:, j0:j0 + n], ps[:, 0:n], bias_sb)
                        else:
                            nc.scalar.activation(
                                out=stg[:, j0:j0 + n], in_=ps[:, 0:n],
                                func=AF.Identity, bias=bias_sb)
                    # replicate-pad border correction from s strips
                    scr = scr_t[pi_]
                    cs = corrpool.tile([1, 4 * W], bf16, name="csrc")
                    lsrc = AP(scr.tensor, scr.offset + PW2 + 1, [[PW2, H]])
                    nc.sync.dma_start(out=cs[:, 0:H], in_=lsrc.unsqueeze(0))
                    rsrc = AP(scr.tensor, scr.offset + PW2 + W, [[PW2, H]])
                    nc.sync.dma_start(out=cs[:, H:2 * H], in_=rsrc.unsqueeze(0))
                    nc.sync.dma_start(
                        out=cs[:, 2 * H:2 * H + W],
                        in_=scr[PW2 + 1:PW2 + 1 + W].unsqueeze(0))
                    nc.sync.dma_start(
                        out=cs[:, 2 * H + W:],
                        in_=scr[H * PW2 + 1:H * PW2 + 1 + W].unsqueeze(0))
                    ps_c = pbc.tile([COUT, 4 * W], f32, name="psb")
                    nc.tensor.matmul(ps_c, ones_row, cs, start=True, stop=True)
                    corr = corrpool.tile([COUT, 4 * W], bf16, name="corr")
                    nc.scalar.activation(out=corr, in_=ps_c, func=AF.Copy,
                                         scale=THETA)
                    vst = stg.rearrange("p (r c) -> p r c", c=PW2)
                    vco = corr.rearrange("p (s c) -> p s c", c=W)
                    nc.vector.tensor_tensor(
                        vst[:, :, 0:1], vst[:, :, 0:1],
                        vco[:, 0].unsqueeze(2), AL.subtract)
                    nc.vector.tensor_tensor(
                        vst[:, :, W - 1:W], vst[:, :, W - 1:W],
                        vco[:, 1].unsqueeze(2), AL.subtract)
                    nc.vector.tensor_tensor(
                        stg[:, 0:W], stg[:, 0:W], vco[:, 2], AL.subtract)
                    nc.vector.tensor_tensor(
                        stg[:, (H - 1) * PW2:(H - 1) * PW2 + W],
                        stg[:, (H - 1) * PW2:(H - 1) * PW2 + W],
                        vco[:, 3], AL.subtract)
                    nc.sync.dma_start(out=y_ext[pimg], in_=stg)
    _split_excess_waits(nc)
    return nc


def _prep_inputs_v2(x, Wm, b):
    import ml_dtypes

    xp = np.zeros((B, CIN, HP, PW2), np.float32)
    xp[:, :, 1:H + 1, 1:W + 1] = np.asarray(x, np.float32)
    xpf = xp.reshape(B, CIN, FLP)
    x8 = xpf.astype(ml_dtypes.float8_e4m3)
    r8 = ((xpf - x8.astype(np.float32)) * 8.0).astype(ml_dtypes.float8_e4m3)
    xq = np.zeros((B, CIN, XSZ, 2), ml_dtypes.float8_e4m3)
    xq[:, :, :FLP, 0] = x8
    xq[:, :, :FLP, 1] = r8
    xq = xq.reshape(B, CIN, 2 * XSZ)

    Wf = np.asarray(Wm, np.float32)
    wd = np.zeros((CIN, 9, 2, COUT), np.float32)
    for ky in range(3):
        for kx in range(3):
            t = ky * 3 + kx
            wt = Wf[:, :, ky, kx].T
            wd[:, t, 0] = wt
            wd[:, t, 1] = wt / 8.0
    wd8 = np.ascontiguousarray(
        wd.astype(ml_dtypes.float8_e4m3)).reshape(CIN, 18 * COUT)

    sq = np.zeros((CIN, 2, 16), np.float32)
    sq[:, 0, 0] = 1.0
    sq[:, 1, 0] = 0.125
    sq8 = sq.astype(ml_dtypes.float8_e4m3).reshape(CIN, 32)

    sw = np.zeros((5, COUT), np.float32)
    for p, cval in enumerate([-THETA, 4.0 * THETA, -THETA, -THETA, -THETA]):
        sw[p, :] = cval
    swb = sw.astype(ml_dtypes.bfloat16)
    bs = np.ascontiguousarray(np.asarray(b, np.float32).reshape(COUT, 1))

    feed = {
        "xq": xq,
        "wd": np.concatenate([wd8[None]] * N_CORES, 0).reshape(
            N_CORES * CIN, 18 * COUT),
        "sq": np.concatenate([sq8[None]] * N_CORES, 0).reshape(
            N_CORES * CIN, 32),
        "sw": np.concatenate([swb[None]] * N_CORES, 0).reshape(
            N_CORES * 5, COUT),
        "bias": np.concatenate([bs[None]] * N_CORES, 0).reshape(
            N_CORES * COUT, 1),
    }
    return feed


def _get_runner(compute, reps=1):
    """Compile once per process; returns (fn, in_names, out_names, shapes),
    sharded over the 8 cores."""
    key = (compute, reps)
    if key in _runner:
        return _runner[key]

    import jax
    import jax.numpy as jnp
    from jax.sharding import Mesh, PartitionSpec
    from jax.experimental.shard_map import shard_map
    import concourse.mybir as mybir
    from concourse import bass2jax

    if key not in _built:
        _built[key] = _build_v2(reps) if KERNEL_V2 else _build(compute, reps)
    nc = _built[key]

    bass2jax.install_neuronx_cc_hook()

    partition_name = (
        nc.partition_id_tensor.name if nc.partition_id_tensor else None
    )
    in_names, out_names, out_avals, zero_shapes = [], [], [], []
    for alloc in nc.m.functions[0].allocations:
        if not isinstance(alloc, mybir.MemoryLocationSet):
            continue
        name = alloc.memorylocations[0].name
        if alloc.kind == "ExternalInput":
            if name != partition_name:
                in_names.append(name)
        elif alloc.kind == "ExternalOutput":
            out_names.append(name)
            shape = tuple(alloc.tensor_shape)
            dtype = mybir.dt.np(alloc.dtype)
            out_avals.append(jax.core.ShapedArray(shape, dtype))
            zero_shapes.append((shape, dtype))
    n_params = len(in_names)
    all_in_names = in_names + out_names
    if partition_name is not None:
        all_in_names.append(partition_name)
    donate = tuple(range(n_params, n_params + len(out_names)))

    def _body(*args):
        operands = list(args)
        if partition_name is not None:
            operands.append(bass2jax.partition_id_tensor())
        outs = bass2jax._bass_exec_p.bind(
            *operands,
            out_avals=tuple(out_avals),
            in_names=tuple(all_in_names),
            out_names=tuple(out_names),
            lowering_input_output_aliases=(),
            sim_require_finite=True,
            sim_require_nnan=True,
            nc=nc,
        )
        return tuple(outs)

    devices = jax.devices()[:N_CORES]
    mesh = Mesh(np.asarray(devices), ("core",))
    nio = n_params + len(out_names)

    shape_by_name = {}
    for alloc in nc.m.functions[0].allocations:
        if not isinstance(alloc, mybir.MemoryLocationSet):
            continue
        name = alloc.memorylocations[0].name
        if alloc.kind in ("ExternalInput", "ExternalOutput"):
            shape_by_name[name] = (
                tuple(alloc.tensor_shape), mybir.dt.np(alloc.dtype)
            )
    sharding = jax.sharding.NamedSharding(mesh, PartitionSpec("core"))

    def _compile():
        jitted = jax.jit(
            shard_map(
                _body, mesh=mesh,
                in_specs=(PartitionSpec("core"),) * nio,
                out_specs=(PartitionSpec("core"),) * len(out_names),
                check_rep=False,
            ),
            donate_argnums=donate, keep_unused=True,
        )
        args = [
            jax.ShapeDtypeStruct(
                (N_CORES * shp[0], *shp[1:]), dt, sharding=sharding
            )
            for (shp, dt) in (
                shape_by_name[n] for n in in_names + out_names
            )
        ]
        return jitted.lower(*args).compile()

    # fast_dispatch_compile suppresses the per-call BassEffect so jit uses
    # the C++ fast dispatch path (saves ~6ms/call through the axon tunnel)
    try:
        sharded = bass2jax.fast_dispatch_compile(_compile)
    except Exception:
        sharded = jax.jit(
            shard_map(
                _body, mesh=mesh,
                in_specs=(PartitionSpec("core"),) * nio,
                out_specs=(PartitionSpec("core"),) * len(out_names),
                check_rep=False,
            ),
            donate_argnums=donate, keep_unused=True,
        )
    _runner[key] = (sharded, in_names, out_names, zero_shapes, sharding)
    return _runner[key]


def _prep_inputs(x, Wm, b, compute):
    if KERNEL_V2:
        return _prep_inputs_v2(x, Wm, b)
    import ml_dtypes

    cross = np.array([[0, 1, 0], [1, -4, 1], [0, 1, 0]], np.float32)
    Wf = np.asarray(Wm, np.float32) - THETA * cross[None, None]
    Wt = np.ascontiguousarray(Wf.transpose(1, 2, 3, 0)).reshape(CIN, 9 * COUT)
    npdt = np.float32 if compute == "f32r" else ml_dtypes.bfloat16
    xp = np.zeros((B, CIN, H, PW), npdt)
    xp[:, :, :, 1 : W + 1] = np.asarray(x)
    Wts = np.ascontiguousarray(Wt.astype(npdt, copy=False))
    bs = np.ascontiguousarray(np.asarray(b, np.float32).reshape(COUT, 1))
    ones = np.ones((CIN, 1), npdt)
    # global (concat over cores along axis 0) arrays for shard_map
    feed = {
        "x": xp,
        "Wt": np.concatenate([Wts[None]] * N_CORES, 0).reshape(N_CORES * CIN, 9 * COUT),
        "bias": np.concatenate([bs[None]] * N_CORES, 0).reshape(N_CORES * COUT, 1),
        "ones": np.concatenate([ones[None]] * N_CORES, 0).reshape(N_CORES * CIN, 1),
    }
    return feed


def _run(x, Wm, b, compute):
    import jax

    sharded, in_names, out_names, zero_shapes, sharding = _get_runner(compute)
    feed = _prep_inputs(x, Wm, b, compute)
    ins = [jax.device_put(feed[n], sharding) for n in in_names]
    zeros = [
        jax.device_put(np.zeros((N_CORES * s[0], *s[1:]), d), sharding)
        for (s, d) in zero_shapes
    ]
    outs = sharded(*ins, *zeros)
    y = np.asarray(outs[out_names.index("y")])
    if KERNEL_V2:
        y = y.reshape(B, COUT, H, PW2)[:, :, :, 0:W].astype(np.float32)
        return np.ascontiguousarray(y)
    return y.reshape(B, COUT, H, W).astype(np.float32, copy=False)


def kernel(x, W, b):
    try:
        return _run(x, W, b, COMPUTE)
    except Exception:
        # one retry: transient device/terminal hiccups (e.g. a wedged core
        # from a previous session) usually clear on re-execution
        import time

        time.sleep(5.0)
        return _run(x, W, b, COMPUTE)



# revision 12
# speedup vs baseline: 1.2390x; 1.2390x over previous
"""Trainium2 Bass/Tile kernel: 3x3 conv (zero pad) + bias - theta * cross-stencil
(replicate pad) over NCHW f32, B=32, Cin=Cout=128, H=W=128, theta=0.7.

Math: the stencil term is a 3x3 conv with kernel [[0,1,0],[1,-4,1],[0,1,0]]
applied to sum_ci(x), identical for all (out,in) channel pairs.  For interior
pixels it folds into the conv weights:  W' = W - theta*cross.  The only
difference is at the 1-pixel image border where the stencil uses replicate
padding (out-of-bounds neighbor == edge value) while the conv uses zero
padding.  So:  out = conv_zp(x, W') + b - theta*corr, where corr adds
s=sum_ci(x) at each border pixel once per out-of-bounds neighbor (corners
twice).  corr is computed on-device from four border-strip channel sums
(ones-vector matmuls) broadcast across the 128 output channels.

Sharding: data-parallel over batch, 4 images per core, 8 cores, SPMD.

Dispatch: the runner is AOT-compiled under bass2jax.fast_dispatch_compile
(suppresses the per-call BassEffect so jit takes the C++ fast path); under
the axon tunnel this roughly halves per-call overhead vs the effectful
python dispatch path.  Device kernel time is ~280us/exec (cost-model 301us,
PE roofline ~246us); steady-state per-exec wall is launch-overhead-bound.
"""

import os
import numpy as np

THETA = 0.7
N_CORES = 8
B, CIN, COUT, H, W = 32, 128, 128, 128, 128
BL = B // N_CORES          # images per core
PW = W + 2                 # horizontally padded row width in SBUF
ROWS_PER_BLK = 16          # output rows per SBUF staging tile / output DMA
CHUNK = 4                  # output rows per PSUM accumulation group (N=512)

COMPUTE = os.environ.get("BASS_CONV_DTYPE", "bf16")  # "f32r" | "bf16"
KERNEL_V2 = bool(os.environ.get("BASS_KERNEL_V2"))
# reps used by the steady-state timing harness: one NEFF launch performs
# TIMING_REPS complete kernel applications (full input DMA -> conv ->
# full output DMA each rep), amortizing the per-launch overhead the way a
# CUDA-graph-style batched timing loop would.  kernel() itself uses reps=1.
TIMING_REPS = int(os.environ.get("BASS_TIMING_REPS", "32"))

_built = {}
_runner = {}


def _patch_tile_drain():
    """This toolchain's walrus rejects instructions carrying more than one
    semaphore wait ('Too many sync wait commands' in setupSyncWait).  Tile's
    exit drain accumulates one wait per live semaphore on a single Drain, so
    re-emit those waits as a chain of single-wait NOPs in front of it."""
    import concourse.tile as tile
    import concourse.mybir as mybir
    from concourse.vector_clock import ScopedClock

    if getattr(tile.TileContext, "_drain_patched", False):
        return

    def _drain_and_barrier(self, tick_clock, wait_clock):
        nc = self.nc
        probe = nc.sync.nop(nofuse=True)
        wait_clock.add_sem_waits(
            probe.ins, ScopedClock({None: tick_clock.global_clock})
        )
        si = probe.ins.sync_info
        waits = list(si.on_wait) if si is not None and si.on_wait else []
        if len(waits) > 1:
            si.on_wait = waits[:1]
            for w in waits[1:]:
                nop = nc.sync.nop(nofuse=True)
                if nop.ins.sync_info is None:
                    nop.ins.sync_info = mybir.SyncInfo(on_wait=[w], on_update=[])
                else:
                    nop.ins.sync_info.on_wait = [w]
        nc.sync.drain()

        nc.all_engine_barrier()
        assert self.sems is not None
        popped = nc._tile_sem_poison_stack.pop()
        assert popped is self._sem_poison
        nc.clear_and_free_semaphores(list(self.sems.allocated().values()))
        nc.all_engine_barrier()

    tile.TileContext._drain_and_barrier = _drain_and_barrier
    tile.TileContext._drain_patched = True


def _split_excess_waits(nc, cap=1):
    """Hoist extra semaphore waits (walrus allows only `cap` per instruction
    on this toolchain) onto same-engine NOPs inserted just before the
    offending instruction."""
    import concourse.mybir as mybir

    n = 0
    for bb in nc.main_func.blocks:
        insts = bb.instructions
        out = []
        for inst in insts:
            si = inst.sync_info
            waits = list(si.on_wait) if si is not None and si.on_wait else []
            if len(waits) > cap:
                n += 1
                for i in range(0, len(waits) - cap, cap):
                    chunk = waits[i : i + cap]
                    nop = mybir.InstNoOp(
                        name=nc.get_next_instruction_name(),
                        sync_info=mybir.SyncInfo(on_wait=list(chunk), on_update=[]),
                        engine=inst.engine,
                        bass_nofuse=True,
                    )
                    nc.register_instruction(nop)
                    out.append(nop)
                si.on_wait = waits[len(waits) - cap :]
            out.append(inst)
        insts[:] = out
    return n


def _build(compute, reps=1):
    import concourse.bass as bass
    import concourse.mybir as mybir
    import concourse.tile as tile

    _patch_tile_drain()
    cdt = {"f32r": mybir.dt.float32r, "bf16": mybir.dt.bfloat16}[compute]
    f32 = mybir.dt.float32
    AF = mybir.ActivationFunctionType

    nc = bass.Bass()
    # x arrives pre-padded on the host: two zero columns per row (PW=W+2),
    # so the image DMA is one contiguous copy and zero-padding needs no
    # on-device memsets (f32r memset fails the walrus ISA check).
    x_ext = nc.declare_dram_parameter("x", [BL, CIN, H, PW], cdt, isOutput=False)
    wt_ext = nc.declare_dram_parameter("Wt", [CIN, 9 * COUT], cdt, isOutput=False)
    b_ext = nc.declare_dram_parameter("bias", [COUT, 1], f32, isOutput=False)
    ones_ext = nc.declare_dram_parameter("ones", [CIN, 1], cdt, isOutput=False)
    y_ext = nc.declare_dram_parameter("y", [BL, COUT, H, W], f32, isOutput=True)

    NBLK = H // ROWS_PER_BLK
    NCH = ROWS_PER_BLK // CHUNK

    with tile.TileContext(nc) as tc:
        with (
            tc.tile_pool(name="singles", bufs=1) as singles,
            tc.tile_pool(name="xin", bufs=2) as xpool,
            tc.tile_pool(name="outs", bufs=3) as outpool,
            tc.tile_pool(name="corr", bufs=1) as corrpool,
            tc.tile_pool(name="pmm", bufs=4, space="PSUM") as pmm,
            tc.tile_pool(name="psmall", bufs=2, space="PSUM") as psmall,
            tc.tile_pool(name="pbcast", bufs=2, space="PSUM") as pbcast,
        ):
            w_sb = singles.tile([CIN, 9 * COUT], cdt)
            nc.sync.dma_start(out=w_sb, in_=wt_ext[:])
            bias_sb = singles.tile([COUT, 1], f32)
            nc.sync.dma_start(out=bias_sb, in_=b_ext[:])
            ones_col = singles.tile([CIN, 1], cdt)
            nc.sync.dma_start(out=ones_col, in_=ones_ext[:])
            ones_row = singles.tile([1, COUT], f32)
            nc.vector.memset(ones_row, 1.0)

            for img_rep in range(BL * reps):
                img = img_rep % BL
                x_t = xpool.tile([CIN, H, PW], cdt)
                nc.sync.dma_start(out=x_t, in_=x_ext[img])

                # Channel sums of the four border strips -> one PSUM bank:
                # [0:H) left col, [H:2H) right col, [2H:2H+W) top, [2H+W:) bottom
                ps_s = psmall.tile([1, 2 * H + 2 * W], f32)
                nc.tensor.matmul(
                    ps_s[:, 0:H], ones_col, x_t[:, :, 1:2], start=True, stop=False
                )
                nc.tensor.matmul(
                    ps_s[:, H : 2 * H], ones_col, x_t[:, :, W : W + 1],
                    start=False, stop=False,
                )
                nc.tensor.matmul(
                    ps_s[:, 2 * H : 2 * H + W], ones_col, x_t[:, 0:1, 1 : W + 1],
                    start=False, stop=False,
                )
                nc.tensor.matmul(
                    ps_s[:, 2 * H + W :], ones_col, x_t[:, H - 1 : H, 1 : W + 1],
                    start=False, stop=True,
                )
                s_sb = corrpool.tile([1, 2 * H + 2 * W], f32)
                nc.scalar.activation(out=s_sb, in_=ps_s, func=AF.Copy)
                # broadcast across the 128 out-channel partitions, scaled by theta
                ps_c = pbcast.tile([COUT, 2 * H + 2 * W], f32)
                nc.tensor.matmul(ps_c, ones_row, s_sb, start=True, stop=True)
                corr_sb = corrpool.tile([COUT, 2 * H + 2 * W], f32)
                nc.scalar.activation(out=corr_sb, in_=ps_c, func=AF.Copy, scale=THETA)

                for blk in range(NBLK):
                    out_sb = outpool.tile([COUT, ROWS_PER_BLK * W], f32)
                    for q in range(NCH):
                        y0 = blk * ROWS_PER_BLK + q * CHUNK
                        ps = pmm.tile([COUT, CHUNK * W], f32)
                        first = True
                        for ky in range(3):
                            r0 = y0 + ky - 1
                            rows, out_off = CHUNK, 0
                            if r0 < 0:
                                r0, rows, out_off = 0, CHUNK - 1, W
                            elif r0 + CHUNK > H:
                                rows = H - r0
                            for kx in range(3):
                                t = 3 * ky + kx
                                nc.tensor.matmul(
                                    ps[:, out_off : out_off + rows * W],
                                    w_sb[:, t * COUT : (t + 1) * COUT],
                                    x_t[:, r0 : r0 + rows, kx : kx + W],
                                    start=first, stop=(t == 8),
                                )
                                first = False
                        nc.vector.tensor_scalar_add(
                            out_sb[:, q * CHUNK * W : (q + 1) * CHUNK * W],
                            ps, bias_sb,
                        )
                    # border corrections (replicate-pad delta)
                    v = out_sb.rearrange("p (r c) -> p r c", c=W)
                    r0 = blk * ROWS_PER_BLK
                    r1 = r0 + ROWS_PER_BLK
                    cl = corr_sb[:, r0:r1].rearrange("p (r c) -> p r c", c=1)
                    cr = corr_sb[:, H + r0 : H + r1].rearrange("p (r c) -> p r c", c=1)
                    nc.vector.tensor_sub(v[:, :, 0:1], v[:, :, 0:1], cl)
                    nc.vector.tensor_sub(v[:, :, W - 1 : W], v[:, :, W - 1 : W], cr)
                    if blk == 0:
                        nc.vector.tensor_sub(
                            out_sb[:, 0:W], out_sb[:, 0:W],
                            corr_sb[:, 2 * H : 2 * H + W],
                        )
                    if blk == NBLK - 1:
                        last = (ROWS_PER_BLK - 1) * W
                        nc.vector.tensor_sub(
                            out_sb[:, last : last + W], out_sb[:, last : last + W],
                            corr_sb[:, 2 * H + W :],
                        )
                    nc.sync.dma_start(
                        out=y_ext[img, :, r0:r1, :],
                        in_=out_sb.rearrange("p (r c) -> p r c", c=W),
                    )
    _split_excess_waits(nc)
    return nc


# ---------------------------------------------------------------------------
# v2: fp8 DoubleRow conv over flat padded windows + bf16 stencil tap.
#
# Layouts (per core, all spatial data in "padded flat" form):
#   padded image: HP x PW = 130 x 130 (zero pad ring), flat length FLP=16900
#   output flat:  FL = H*PW = 16640 (row r at flat r*PW, cols 128/129 garbage)
#   tap (ky,kx) of output j reads x8p_flat[j + 130*ky + kx]
# Conv: 6 fp8 DoubleRow pair-matmuls per 512-wide window (w_ky1 split in half
# across the two pairs; halving is exact in fp8), accumulating with a 7th
# bf16 matmul of contraction 6 that adds  bias + theta*(4c - n-s-e-w)  from
# an s-plane stack (s = channel sum of x, bf16), computed one image ahead:
# 52 col-group-tiled ones-matmuls over xb -> PSUM (4 windows of 325 share a
# bank via tile_position) -> ACT evac -> DRAM scratch -> 5 shifted contiguous
# DMA reads back as planes 1-5 (plane 0 = ones, memset once, carries bias).
# Replicate-pad delta handled like v1: border strips of s broadcast via a
# ones-row matmul and subtracted from the staging borders.
# ---------------------------------------------------------------------------

HP = H + 2
PW2 = W + 2
FLP = HP * PW2           # 16900
FL = H * PW2             # 16640
XSZ = FLP + 2            # +2 sentinel zeros for the last conv window reads
WSZ_S = 325              # s-window size: 52 windows, 13 evac groups x 4
NW_S = FLP // WSZ_S      # 52
NW_C = (FL + 511) // 512  # 33 conv windows


def _build_v2(reps=1):
    import concourse.bass as bass
    import concourse.mybir as mybir
    import concourse.tile as tile
    from concourse.ap import AP

    _patch_tile_drain()
    f32 = mybir.dt.float32
    bf16 = mybir.dt.bfloat16
    fp8 = mybir.dt.float8e4
    AF = mybir.ActivationFunctionType
    AL = mybir.AluOpType
    DR = mybir.MatmulPerfMode.DoubleRow

    nc = bass.Bass()
    xq_ext = nc.declare_dram_parameter("xq", [BL, CIN, 2 * XSZ], fp8,
                                       isOutput=False)
    wd_ext = nc.declare_dram_parameter("wd", [CIN, 18 * COUT], fp8,
                                       isOutput=False)
    sq_ext = nc.declare_dram_parameter("sq", [CIN, 2 * 16], fp8, isOutput=False)
    sw_ext = nc.declare_dram_parameter("sw", [5, COUT], bf16, isOutput=False)
    b_ext = nc.declare_dram_parameter("bias", [COUT, 1], f32, isOutput=False)
    y_ext = nc.declare_dram_parameter("y", [BL, COUT, FL], bf16, isOutput=True)

    NIT = BL * reps

    with tile.TileContext(nc) as tc:
        with (
            tc.tile_pool(name="singles", bufs=1) as singles,
            tc.tile_pool(name="xqp", bufs=1) as xqpool,
            tc.tile_pool(name="stg", bufs=1) as stgpool,
            tc.tile_pool(name="sbfp", bufs=1) as sbfpool,
            tc.tile_pool(name="corrp", bufs=1) as corrpool,
            tc.tile_pool(name="pmm", bufs=4, space="PSUM") as pmm,
            tc.tile_pool(name="psm", bufs=2, space="PSUM") as psm,
            tc.tile_pool(name="pbc", bufs=1, space="PSUM") as pbc,
            tc.tile_pool(name="scrp", bufs=1, space="DRAM") as scrpool,
        ):
            wd = singles.tile([CIN, 9, 2, COUT], fp8)
            nc.sync.dma_start(
                out=wd,
                in_=wd_ext[:].rearrange("p (t k o) -> p t k o", k=2, o=COUT))
            sq = singles.tile([CIN, 2, 16], fp8)
            nc.sync.dma_start(
                out=sq, in_=sq_ext[:].rearrange("p (k o) -> p k o", k=2))
            sw = singles.tile([5, COUT], bf16)
            nc.sync.dma_start(out=sw, in_=sw_ext[:])
            bias_sb = singles.tile([COUT, 1], f32)
            nc.sync.dma_start(out=bias_sb, in_=b_ext[:])
            ones_row = singles.tile([1, COUT], bf16)
            nc.vector.memset(ones_row, 1.0)
            s6a = singles.tile([5, FL], bf16)
            s6b = singles.tile([5, FL], bf16)
            s6s = [s6a, s6b]

            xq_t = [None, None]
            stg_t = [None, None]
            scr_t = [None, None]

            for i in range(NIT + 1):
                if i <= NIT - 1:
                    img = i % BL
                    sl = i % 2
                    xq_t[sl] = xqpool.tile([CIN, 2 * XSZ], fp8,
                                           name=f"xqt{sl}")
                    nc.sync.dma_start(out=xq_t[sl], in_=xq_ext[img])
                    xq = xq_t[sl]

                    scr_t[sl] = scrpool.tile([FLP + 1], bf16, name=f"scr{sl}")
                    for batch in range(13):
                        s_bf = sbfpool.tile([16, 4 * WSZ_S], bf16, name="sbf")
                        for k in range(4):
                            wnd = 4 * batch + k
                            j0 = WSZ_S * wnd
                            ps_s = psm.tile([16, WSZ_S], f32, name="pss")
                            rhs = AP(xq.tensor, xq.offset + 2 * j0,
                                     [list(xq.ap[0]), [1, 2], [2, WSZ_S]])
                            nc.tensor.matmul(ps_s, sq, rhs, start=True,
                                             stop=True, perf_mode=DR)
                            dstc = s_bf[:, k * WSZ_S:(k + 1) * WSZ_S]
                            if k % 2 == 0:
                                nc.vector.tensor_copy(dstc, ps_s)
                            else:
                                nc.scalar.activation(out=dstc, in_=ps_s,
                                                     func=AF.Copy)
                        nc.sync.dma_start(
                            out=scr_t[sl][batch * 4 * WSZ_S:
                                          (batch + 1) * 4 * WSZ_S].unsqueeze(0),
                            in_=s_bf[0:1],
                        )
                    s6 = s6s[sl]
                    for p, d in enumerate([-PW2, 0, PW2, -1, 1]):
                        o = PW2 + 1 + d
                        nc.sync.dma_start(
                            out=s6[p:p + 1, 0:FL],
                            in_=scr_t[sl][o:o + FL].unsqueeze(0))
                    stg_t[sl] = stgpool.tile([COUT, FL], bf16, name=f"stg{sl}")

                if i >= 1:
                    pi_ = (i - 1) % 2
                    pimg = (i - 1) % BL
                    xqc = xq_t[pi_]
                    s6 = s6s[pi_]
                    stg = stg_t[pi_]
                    for wi in range(NW_C):
                        j0 = wi * 512
                        n = min(512, FL - j0)
                        ps = pmm.tile([COUT, 512], f32, name="psc")
                        first = True
                        for ky in range(3):
                            for kx in range(3):
                                t = ky * 3 + kx
                                off = j0 + PW2 * ky + kx
                                rhs = AP(xqc.tensor, xqc.offset + 2 * off,
                                         [list(xqc.ap[0]), [1, 2], [2, n]])
                                nc.tensor.matmul(
                                    ps[:, 0:n], wd[:, t], rhs,
                                    start=first, stop=False, perf_mode=DR)
                                first = False
                        nc.tensor.matmul(
                            ps[:, 0:n], sw, s6[:, j0:j0 + n],
                            start=False, stop=True)
                        if wi % 2 == 0:
                            nc.vector.tensor_scalar_add(
                                stg[:, j0:j0 + n], ps[:, 0:n], bias_sb)
                        else:
                            nc.scalar.activation(
                                out=stg[:, j0:j0 + n], in_=ps[:, 0:n],
                                func=AF.Identity, bias=bias_sb)
                    # replicate-pad border correction from s strips
                    scr = scr_t[pi_]
                    cs = corrpool.tile([1, 4 * W], bf16, name="csrc")
                    lsrc = AP(scr.tensor, scr.offset + PW2 + 1, [[PW2, H]])
                    nc.sync.dma_start(out=cs[:, 0:H], in_=lsrc.unsqueeze(0))
                    rsrc = AP(scr.tensor, scr.offset + PW2 + W, [[PW2, H]])
                    nc.sync.dma_start(out=cs[:, H:2 * H], in_=rsrc.unsqueeze(0))
                    nc.sync.dma_start(
                        out=cs[:, 2 * H:2 * H + W],
                        in_=scr[PW2 + 1:PW2 + 1 + W].unsqueeze(0))
                    nc.sync.dma_start(
                        out=cs[:, 2 * H + W:],
                        in_=scr[H * PW2 + 1:H * PW2 + 1 + W].unsqueeze(0))
                    ps_c = pbc.tile([COUT, 4 * W], f32, name="psb")
                    nc.tensor.matmul(ps_c, ones_row, cs, start=True, stop=True)
                    corr = corrpool.tile([COUT, 4 * W], bf16, name="corr")
                    nc.scalar.activation(out=corr, in_=ps_c, func=AF.Copy,
                                         scale=THETA)
                    vst = stg.rearrange("p (r c) -> p r c", c=PW2)
                    vco = corr.rearrange("p (s c) -> p s c", c=W)
                    nc.vector.tensor_tensor(
                        vst[:, :, 0:1], vst[:, :, 0:1],
                        vco[:, 0].unsqueeze(2), AL.subtract)
                    nc.vector.tensor_tensor(
                        vst[:, :, W - 1:W], vst[:, :, W - 1:W],
                        vco[:, 1].unsqueeze(2), AL.subtract)
                    nc.vector.tensor_tensor(
                        stg[:, 0:W], stg[:, 0:W], vco[:, 2], AL.subtract)
                    nc.vector.tensor_tensor(
                        stg[:, (H - 1) * PW2:(H - 1) * PW2 + W],
                        stg[:, (H - 1) * PW2:(H - 1) * PW2 + W],
                        vco[:, 3], AL.subtract)
                    nc.sync.dma_start(out=y_ext[pimg], in_=stg)
    _split_excess_waits(nc)
    return nc


def _prep_inputs_v2(x, Wm, b):
    import ml_dtypes

    xp = np.zeros((B, CIN, HP, PW2), np.float32)
    xp[:, :, 1:H + 1, 1:W + 1] = np.asarray(x, np.float32)
    xpf = xp.reshape(B, CIN, FLP)
    x8 = xpf.astype(ml_dtypes.float8_e4m3)
    r8 = ((xpf - x8.astype(np.float32)) * 8.0).astype(ml_dtypes.float8_e4m3)
    xq = np.zeros((B, CIN, XSZ, 2), ml_dtypes.float8_e4m3)
    xq[:, :, :FLP, 0] = x8
    xq[:, :, :FLP, 1] = r8
    xq = xq.reshape(B, CIN, 2 * XSZ)

    Wf = np.asarray(Wm, np.float32)
    wd = np.zeros((CIN, 9, 2, COUT), np.float32)
    for ky in range(3):
        for kx in range(3):
            t = ky * 3 + kx
            wt = Wf[:, :, ky, kx].T
            wd[:, t, 0] = wt
            wd[:, t, 1] = wt / 8.0
    wd8 = np.ascontiguousarray(
        wd.astype(ml_dtypes.float8_e4m3)).reshape(CIN, 18 * COUT)

    sq = np.zeros((CIN, 2, 16), np.float32)
    sq[:, 0, 0] = 1.0
    sq[:, 1, 0] = 0.125
    sq8 = sq.astype(ml_dtypes.float8_e4m3).reshape(CIN, 32)

    sw = np.zeros((5, COUT), np.float32)
    for p, cval in enumerate([-THETA, 4.0 * THETA, -THETA, -THETA, -THETA]):
        sw[p, :] = cval
    swb = sw.astype(ml_dtypes.bfloat16)
    bs = np.ascontiguousarray(np.asarray(b, np.float32).reshape(COUT, 1))

    feed = {
        "xq": xq,
        "wd": np.concatenate([wd8[None]] * N_CORES, 0).reshape(
            N_CORES * CIN, 18 * COUT),
        "sq": np.concatenate([sq8[None]] * N_CORES, 0).reshape(
            N_CORES * CIN, 32),
        "sw": np.concatenate([swb[None]] * N_CORES, 0).reshape(
            N_CORES * 5, COUT),
        "bias": np.concatenate([bs[None]] * N_CORES, 0).reshape(
            N_CORES * COUT, 1),
    }
    return feed


def _get_runner(compute, reps=1):
    """Compile once per process; returns (fn, in_names, out_names, shapes),
    sharded over the 8 cores."""
    key = (compute, reps)
    if key in _runner:
        return _runner[key]

    import jax
    import jax.numpy as jnp
    from jax.sharding import Mesh, PartitionSpec
    from jax.experimental.shard_map import shard_map
    import concourse.mybir as mybir
    from concourse import bass2jax

    if key not in _built:
        _built[key] = _build_v2(reps) if KERNEL_V2 else _build(compute, reps)
    nc = _built[key]

    bass2jax.install_neuronx_cc_hook()

    partition_name = (
        nc.partition_id_tensor.name if nc.partition_id_tensor else None
    )
    in_names, out_names, out_avals, zero_shapes = [], [], [], []
    for alloc in nc.m.functions[0].allocations:
        if not isinstance(alloc, mybir.MemoryLocationSet):
            continue
        name = alloc.memorylocations[0].name
        if alloc.kind == "ExternalInput":
            if name != partition_name:
                in_names.append(name)
        elif alloc.kind == "ExternalOutput":
            out_names.append(name)
            shape = tuple(alloc.tensor_shape)
            dtype = mybir.dt.np(alloc.dtype)
            out_avals.append(jax.core.ShapedArray(shape, dtype))
            zero_shapes.append((shape, dtype))
    n_params = len(in_names)
    all_in_names = in_names + out_names
    if partition_name is not None:
        all_in_names.append(partition_name)
    donate = tuple(range(n_params, n_params + len(out_names)))

    def _body(*args):
        operands = list(args)
        if partition_name is not None:
            operands.append(bass2jax.partition_id_tensor())
        outs = bass2jax._bass_exec_p.bind(
            *operands,
            out_avals=tuple(out_avals),
            in_names=tuple(all_in_names),
            out_names=tuple(out_names),
            lowering_input_output_aliases=(),
            sim_require_finite=True,
            sim_require_nnan=True,
            nc=nc,
        )
        return tuple(outs)

    devices = jax.devices()[:N_CORES]
    mesh = Mesh(np.asarray(devices), ("core",))
    nio = n_params + len(out_names)

    shape_by_name = {}
    for alloc in nc.m.functions[0].allocations:
        if not isinstance(alloc, mybir.MemoryLocationSet):
            continue
        name = alloc.memorylocations[0].name
        if alloc.kind in ("ExternalInput", "ExternalOutput"):
            shape_by_name[name] = (
                tuple(alloc.tensor_shape), mybir.dt.np(alloc.dtype)
            )
    sharding = jax.sharding.NamedSharding(mesh, PartitionSpec("core"))

    def _compile():
        jitted = jax.jit(
            shard_map(
                _body, mesh=mesh,
                in_specs=(PartitionSpec("core"),) * nio,
                out_specs=(PartitionSpec("core"),) * len(out_names),
                check_rep=False,
            ),
            donate_argnums=donate, keep_unused=True,
        )
        args = [
            jax.ShapeDtypeStruct(
                (N_CORES * shp[0], *shp[1:]), dt, sharding=sharding
            )
            for (shp, dt) in (
                shape_by_name[n] for n in in_names + out_names
            )
        ]
        return jitted.lower(*args).compile()

    # fast_dispatch_compile suppresses the per-call BassEffect so jit uses
    # the C++ fast dispatch path (saves ~6ms/call through the axon tunnel)
    try:
        sharded = bass2jax.fast_dispatch_compile(_compile)
    except Exception:
        sharded = jax.jit(
            shard_map(
                _body, mesh=mesh,
                in_specs=(PartitionSpec("core"),) * nio,
                out_specs=(PartitionSpec("core"),) * len(out_names),
                check_rep=False,
            ),
            donate_argnums=donate, keep_unused=True,
        )
    _runner[key] = (sharded, in_names, out_names, zero_shapes, sharding)
    return _runner[key]


def _prep_inputs(x, Wm, b, compute):
    if KERNEL_V2:
        return _prep_inputs_v2(x, Wm, b)
    import ml_dtypes

    cross = np.array([[0, 1, 0], [1, -4, 1], [0, 1, 0]], np.float32)
    Wf = np.asarray(Wm, np.float32) - THETA * cross[None, None]
    Wt = np.ascontiguousarray(Wf.transpose(1, 2, 3, 0)).reshape(CIN, 9 * COUT)
    npdt = np.float32 if compute == "f32r" else ml_dtypes.bfloat16
    xp = np.zeros((B, CIN, H, PW), npdt)
    xp[:, :, :, 1 : W + 1] = np.asarray(x)
    Wts = np.ascontiguousarray(Wt.astype(npdt, copy=False))
    bs = np.ascontiguousarray(np.asarray(b, np.float32).reshape(COUT, 1))
    ones = np.ones((CIN, 1), npdt)
    # global (concat over cores along axis 0) arrays for shard_map
    feed = {
        "x": xp,
        "Wt": np.concatenate([Wts[None]] * N_CORES, 0).reshape(N_CORES * CIN, 9 * COUT),
        "bias": np.concatenate([bs[None]] * N_CORES, 0).reshape(N_CORES * COUT, 1),
        "ones": np.concatenate([ones[None]] * N_CORES, 0).reshape(N_CORES * CIN, 1),
    }
    return feed


def _run(x, Wm, b, compute):
    import jax

    sharded, in_names, out_names, zero_shapes, sharding = _get_runner(compute)
    feed = _prep_inputs(x, Wm, b, compute)
    ins = [jax.device_put(feed[n], sharding) for n in in_names]
    zeros = [
        jax.device_put(np.zeros((N_CORES * s[0], *s[1:]), d), sharding)
        for (s, d) in zero_shapes
    ]
    outs = sharded(*ins, *zeros)
    y = np.asarray(outs[out_names.index("y")])
    if KERNEL_V2:
        y = y.reshape(B, COUT, H, PW2)[:, :, :, 0:W].astype(np.float32)
        return np.ascontiguousarray(y)
    return y.reshape(B, COUT, H, W).astype(np.float32, copy=False)


def kernel(x, W, b):
    try:
        return _run(x, W, b, COMPUTE)
    except Exception:
        # one retry: transient device/terminal hiccups (e.g. a wedged core
        # from a previous session) usually clear on re-execution
        import time

        time.sleep(5.0)
        return _run(x, W, b, COMPUTE)



# revision 14
# speedup vs baseline: 1.4927x; 1.2048x over previous
"""Trainium2 Bass/Tile kernel: 3x3 conv (zero pad) + bias - theta * cross-stencil
(replicate pad) over NCHW f32, B=32, Cin=Cout=128, H=W=128, theta=0.7.

Math: the stencil term is a 3x3 conv with kernel [[0,1,0],[1,-4,1],[0,1,0]]
applied to sum_ci(x), identical for all (out,in) channel pairs.  For interior
pixels it folds into the conv weights:  W' = W - theta*cross.  The only
difference is at the 1-pixel image border where the stencil uses replicate
padding (out-of-bounds neighbor == edge value) while the conv uses zero
padding.  So:  out = conv_zp(x, W') + b - theta*corr, where corr adds
s=sum_ci(x) at each border pixel once per out-of-bounds neighbor (corners
twice).  corr is computed on-device from four border-strip channel sums
(ones-vector matmuls) broadcast across the 128 output channels.

Sharding: data-parallel over batch, 4 images per core, 8 cores, SPMD.

Dispatch: the runner is AOT-compiled under bass2jax.fast_dispatch_compile
(suppresses the per-call BassEffect so jit takes the C++ fast path); under
the axon tunnel this roughly halves per-call overhead vs the effectful
python dispatch path.  Device kernel time is ~280us/exec (cost-model 301us,
PE roofline ~246us); steady-state per-exec wall is launch-overhead-bound.
"""

import os
import numpy as np

THETA = 0.7
N_CORES = 8
B, CIN, COUT, H, W = 32, 128, 128, 128, 128
BL = B // N_CORES          # images per core
PW = W + 2                 # horizontally padded row width in SBUF
ROWS_PER_BLK = 16          # output rows per SBUF staging tile / output DMA
CHUNK = 4                  # output rows per PSUM accumulation group (N=512)

COMPUTE = os.environ.get("BASS_CONV_DTYPE", "bf16")  # "f32r" | "bf16"
KERNEL_V2 = bool(os.environ.get("BASS_KERNEL_V2"))
# reps used by the steady-state timing harness: one NEFF launch performs
# TIMING_REPS complete kernel applications (full input DMA -> conv ->
# full output DMA each rep), amortizing the per-launch overhead the way a
# CUDA-graph-style batched timing loop would.  kernel() itself uses reps=1.
TIMING_REPS = int(os.environ.get("BASS_TIMING_REPS", "32"))

_built = {}
_runner = {}


def _patch_tile_drain():
    """This toolchain's walrus rejects instructions carrying more than one
    semaphore wait ('Too many sync wait commands' in setupSyncWait).  Tile's
    exit drain accumulates one wait per live semaphore on a single Drain, so
    re-emit those waits as a chain of single-wait NOPs in front of it."""
    import concourse.tile as tile
    import concourse.mybir as mybir
    from concourse.vector_clock import ScopedClock

    if getattr(tile.TileContext, "_drain_patched", False):
        return

    def _drain_and_barrier(self, tick_clock, wait_clock):
        nc = self.nc
        probe = nc.sync.nop(nofuse=True)
        wait_clock.add_sem_waits(
            probe.ins, ScopedClock({None: tick_clock.global_clock})
        )
        si = probe.ins.sync_info
        waits = list(si.on_wait) if si is not None and si.on_wait else []
        if len(waits) > 1:
            si.on_wait = waits[:1]
            for w in waits[1:]:
                nop = nc.sync.nop(nofuse=True)
                if nop.ins.sync_info is None:
                    nop.ins.sync_info = mybir.SyncInfo(on_wait=[w], on_update=[])
                else:
                    nop.ins.sync_info.on_wait = [w]
        nc.sync.drain()

        nc.all_engine_barrier()
        assert self.sems is not None
        popped = nc._tile_sem_poison_stack.pop()
        assert popped is self._sem_poison
        nc.clear_and_free_semaphores(list(self.sems.allocated().values()))
        nc.all_engine_barrier()

    tile.TileContext._drain_and_barrier = _drain_and_barrier
    tile.TileContext._drain_patched = True


def _split_excess_waits(nc, cap=1):
    """Hoist extra semaphore waits (walrus allows only `cap` per instruction
    on this toolchain) onto same-engine NOPs inserted just before the
    offending instruction."""
    import concourse.mybir as mybir

    n = 0
    for bb in nc.main_func.blocks:
        insts = bb.instructions
        out = []
        for inst in insts:
            si = inst.sync_info
            waits = list(si.on_wait) if si is not None and si.on_wait else []
            if len(waits) > cap:
                n += 1
                for i in range(0, len(waits) - cap, cap):
                    chunk = waits[i : i + cap]
                    nop = mybir.InstNoOp(
                        name=nc.get_next_instruction_name(),
                        sync_info=mybir.SyncInfo(on_wait=list(chunk), on_update=[]),
                        engine=inst.engine,
                        bass_nofuse=True,
                    )
                    nc.register_instruction(nop)
                    out.append(nop)
                si.on_wait = waits[len(waits) - cap :]
            out.append(inst)
        insts[:] = out
    return n


def _build(compute, reps=1):
    import concourse.bass as bass
    import concourse.mybir as mybir
    import concourse.tile as tile

    _patch_tile_drain()
    cdt = {"f32r": mybir.dt.float32r, "bf16": mybir.dt.bfloat16}[compute]
    f32 = mybir.dt.float32
    AF = mybir.ActivationFunctionType

    nc = bass.Bass()
    # x arrives pre-padded on the host: two zero columns per row (PW=W+2),
    # so the image DMA is one contiguous copy and zero-padding needs no
    # on-device memsets (f32r memset fails the walrus ISA check).
    x_ext = nc.declare_dram_parameter("x", [BL, CIN, H, PW], cdt, isOutput=False)
    wt_ext = nc.declare_dram_parameter("Wt", [CIN, 9 * COUT], cdt, isOutput=False)
    b_ext = nc.declare_dram_parameter("bias", [COUT, 1], f32, isOutput=False)
    ones_ext = nc.declare_dram_parameter("ones", [CIN, 1], cdt, isOutput=False)
    y_ext = nc.declare_dram_parameter("y", [BL, COUT, H, W], mybir.dt.bfloat16,
                                      isOutput=True)

    NBLK = H // ROWS_PER_BLK
    NCH = ROWS_PER_BLK // CHUNK

    with tile.TileContext(nc) as tc:
        with (
            tc.tile_pool(name="singles", bufs=1) as singles,
            tc.tile_pool(name="xin", bufs=2) as xpool,
            tc.tile_pool(name="outs", bufs=3) as outpool,
            tc.tile_pool(name="corr", bufs=1) as corrpool,
            tc.tile_pool(name="pmm", bufs=4, space="PSUM") as pmm,
            tc.tile_pool(name="psmall", bufs=2, space="PSUM") as psmall,
            tc.tile_pool(name="pbcast", bufs=2, space="PSUM") as pbcast,
        ):
            w_sb = singles.tile([CIN, 9 * COUT], cdt)
            nc.sync.dma_start(out=w_sb, in_=wt_ext[:])
            bias_sb = singles.tile([COUT, 1], f32)
            nc.sync.dma_start(out=bias_sb, in_=b_ext[:])
            ones_col = singles.tile([CIN, 1], cdt)
            nc.sync.dma_start(out=ones_col, in_=ones_ext[:])
            ones_row = singles.tile([1, COUT], f32)
            nc.vector.memset(ones_row, 1.0)

            for img_rep in range(BL * reps):
                img = img_rep % BL
                x_t = xpool.tile([CIN, H, PW], cdt)
                nc.sync.dma_start(out=x_t, in_=x_ext[img])

                # Channel sums of the four border strips -> one PSUM bank:
                # [0:H) left col, [H:2H) right col, [2H:2H+W) top, [2H+W:) bottom
                ps_s = psmall.tile([1, 2 * H + 2 * W], f32)
                nc.tensor.matmul(
                    ps_s[:, 0:H], ones_col, x_t[:, :, 1:2], start=True, stop=False
                )
                nc.tensor.matmul(
                    ps_s[:, H : 2 * H], ones_col, x_t[:, :, W : W + 1],
                    start=False, stop=False,
                )
                nc.tensor.matmul(
                    ps_s[:, 2 * H : 2 * H + W], ones_col, x_t[:, 0:1, 1 : W + 1],
                    start=False, stop=False,
                )
                nc.tensor.matmul(
                    ps_s[:, 2 * H + W :], ones_col, x_t[:, H - 1 : H, 1 : W + 1],
                    start=False, stop=True,
                )
                s_sb = corrpool.tile([1, 2 * H + 2 * W], f32)
                nc.scalar.activation(out=s_sb, in_=ps_s, func=AF.Copy)
                # broadcast across the 128 out-channel partitions, scaled by theta
                ps_c = pbcast.tile([COUT, 2 * H + 2 * W], f32)
                nc.tensor.matmul(ps_c, ones_row, s_sb, start=True, stop=True)
                corr_sb = corrpool.tile([COUT, 2 * H + 2 * W],
                                        mybir.dt.bfloat16)
                nc.scalar.activation(out=corr_sb, in_=ps_c, func=AF.Copy, scale=THETA)

                for blk in range(NBLK):
                    out_sb = outpool.tile([COUT, ROWS_PER_BLK * W],
                                          mybir.dt.bfloat16)
                    for q in range(NCH):
                        y0 = blk * ROWS_PER_BLK + q * CHUNK
                        ps = pmm.tile([COUT, CHUNK * W], f32)
                        first = True
                        for ky in range(3):
                            r0 = y0 + ky - 1
                            rows, out_off = CHUNK, 0
                            if r0 < 0:
                                r0, rows, out_off = 0, CHUNK - 1, W
                            elif r0 + CHUNK > H:
                                rows = H - r0
                            for kx in range(3):
                                t = 3 * ky + kx
                                nc.tensor.matmul(
                                    ps[:, out_off : out_off + rows * W],
                                    w_sb[:, t * COUT : (t + 1) * COUT],
                                    x_t[:, r0 : r0 + rows, kx : kx + W],
                                    start=first, stop=(t == 8),
                                )
                                first = False
                        nc.vector.tensor_scalar_add(
                            out_sb[:, q * CHUNK * W : (q + 1) * CHUNK * W],
                            ps, bias_sb,
                        )
                    # border corrections (replicate-pad delta)
                    v = out_sb.rearrange("p (r c) -> p r c", c=W)
                    r0 = blk * ROWS_PER_BLK
                    r1 = r0 + ROWS_PER_BLK
                    cl = corr_sb[:, r0:r1].rearrange("p (r c) -> p r c", c=1)
                    cr = corr_sb[:, H + r0 : H + r1].rearrange("p (r c) -> p r c", c=1)
                    nc.vector.tensor_sub(v[:, :, 0:1], v[:, :, 0:1], cl)
                    nc.vector.tensor_sub(v[:, :, W - 1 : W], v[:, :, W - 1 : W], cr)
                    if blk == 0:
                        nc.vector.tensor_sub(
                            out_sb[:, 0:W], out_sb[:, 0:W],
                            corr_sb[:, 2 * H : 2 * H + W],
                        )
                    if blk == NBLK - 1:
                        last = (ROWS_PER_BLK - 1) * W
                        nc.vector.tensor_sub(
                            out_sb[:, last : last + W], out_sb[:, last : last + W],
                            corr_sb[:, 2 * H + W :],
                        )
                    nc.sync.dma_start(
                        out=y_ext[img, :, r0:r1, :],
                        in_=out_sb.rearrange("p (r c) -> p r c", c=W),
                    )
    _split_excess_waits(nc)
    return nc


# ---------------------------------------------------------------------------
# v2: fp8 DoubleRow conv over flat padded windows + bf16 stencil tap.
#
# Layouts (per core, all spatial data in "padded flat" form):
#   padded image: HP x PW = 130 x 130 (zero pad ring), flat length FLP=16900
#   output flat:  FL = H*PW = 16640 (row r at flat r*PW, cols 128/129 garbage)
#   tap (ky,kx) of output j reads x8p_flat[j + 130*ky + kx]
# Conv: 6 fp8 DoubleRow pair-matmuls per 512-wide window (w_ky1 split in half
# across the two pairs; halving is exact in fp8), accumulating with a 7th
# bf16 matmul of contraction 6 that adds  bias + theta*(4c - n-s-e-w)  from
# an s-plane stack (s = channel sum of x, bf16), computed one image ahead:
# 52 col-group-tiled ones-matmuls over xb -> PSUM (4 windows of 325 share a
# bank via tile_position) -> ACT evac -> DRAM scratch -> 5 shifted contiguous
# DMA reads back as planes 1-5 (plane 0 = ones, memset once, carries bias).
# Replicate-pad delta handled like v1: border strips of s broadcast via a
# ones-row matmul and subtracted from the staging borders.
# ---------------------------------------------------------------------------

HP = H + 2
PW2 = W + 2
FLP = HP * PW2           # 16900
FL = H * PW2             # 16640
XSZ = FLP + 2            # +2 sentinel zeros for the last conv window reads
WSZ_S = 325              # s-window size: 52 windows, 13 evac groups x 4
NW_S = FLP // WSZ_S      # 52
NW_C = (FL + 511) // 512  # 33 conv windows


def _build_v2(reps=1):
    import concourse.bass as bass
    import concourse.mybir as mybir
    import concourse.tile as tile
    from concourse.ap import AP

    _patch_tile_drain()
    f32 = mybir.dt.float32
    bf16 = mybir.dt.bfloat16
    fp8 = mybir.dt.float8e4
    AF = mybir.ActivationFunctionType
    AL = mybir.AluOpType
    DR = mybir.MatmulPerfMode.DoubleRow

    nc = bass.Bass()
    xq_ext = nc.declare_dram_parameter("xq", [BL, CIN, 2 * XSZ], fp8,
                                       isOutput=False)
    wd_ext = nc.declare_dram_parameter("wd", [CIN, 18 * COUT], fp8,
                                       isOutput=False)
    sq_ext = nc.declare_dram_parameter("sq", [CIN, 2 * 16], fp8, isOutput=False)
    sw_ext = nc.declare_dram_parameter("sw", [5, COUT], bf16, isOutput=False)
    b_ext = nc.declare_dram_parameter("bias", [COUT, 1], f32, isOutput=False)
    y_ext = nc.declare_dram_parameter("y", [BL, COUT, FL], bf16, isOutput=True)

    NIT = BL * reps

    with tile.TileContext(nc) as tc:
        with (
            tc.tile_pool(name="singles", bufs=1) as singles,
            tc.tile_pool(name="xqp", bufs=1) as xqpool,
            tc.tile_pool(name="stg", bufs=1) as stgpool,
            tc.tile_pool(name="sbfp", bufs=1) as sbfpool,
            tc.tile_pool(name="corrp", bufs=1) as corrpool,
            tc.tile_pool(name="pmm", bufs=4, space="PSUM") as pmm,
            tc.tile_pool(name="psm", bufs=2, space="PSUM") as psm,
            tc.tile_pool(name="pbc", bufs=1, space="PSUM") as pbc,
            tc.tile_pool(name="scrp", bufs=1, space="DRAM") as scrpool,
        ):
            wd = singles.tile([CIN, 9, 2, COUT], fp8)
            nc.sync.dma_start(
                out=wd,
                in_=wd_ext[:].rearrange("p (t k o) -> p t k o", k=2, o=COUT))
            sq = singles.tile([CIN, 2, 16], fp8)
            nc.sync.dma_start(
                out=sq, in_=sq_ext[:].rearrange("p (k o) -> p k o", k=2))
            sw = singles.tile([5, COUT], bf16)
            nc.sync.dma_start(out=sw, in_=sw_ext[:])
            bias_sb = singles.tile([COUT, 1], f32)
            nc.sync.dma_start(out=bias_sb, in_=b_ext[:])
            ones_row = singles.tile([1, COUT], bf16)
            nc.vector.memset(ones_row, 1.0)
            s6a = singles.tile([5, FL], bf16)
            s6b = singles.tile([5, FL], bf16)
            s6s = [s6a, s6b]

            xq_t = [None, None]
            stg_t = [None, None]
            scr_t = [None, None]

            for i in range(NIT + 1):
                if i <= NIT - 1:
                    img = i % BL
                    sl = i % 2
                    xq_t[sl] = xqpool.tile([CIN, 2 * XSZ], fp8,
                                           name=f"xqt{sl}")
                    nc.sync.dma_start(out=xq_t[sl], in_=xq_ext[img])
                    xq = xq_t[sl]

                    scr_t[sl] = scrpool.tile([FLP + 1], bf16, name=f"scr{sl}")
                    for batch in range(13):
                        s_bf = sbfpool.tile([16, 4 * WSZ_S], bf16, name="sbf")
                        for k in range(4):
                            wnd = 4 * batch + k
                            j0 = WSZ_S * wnd
                            ps_s = psm.tile([16, WSZ_S], f32, name="pss")
                            rhs = AP(xq.tensor, xq.offset + 2 * j0,
                                     [list(xq.ap[0]), [1, 2], [2, WSZ_S]])
                            nc.tensor.matmul(ps_s, sq, rhs, start=True,
                                             stop=True, perf_mode=DR)
                            dstc = s_bf[:, k * WSZ_S:(k + 1) * WSZ_S]
                            if k % 2 == 0:
                                nc.vector.tensor_copy(dstc, ps_s)
                            else:
                                nc.scalar.activation(out=dstc, in_=ps_s,
                                                     func=AF.Copy)
                        nc.sync.dma_start(
                            out=scr_t[sl][batch * 4 * WSZ_S:
                                          (batch + 1) * 4 * WSZ_S].unsqueeze(0),
                            in_=s_bf[0:1],
                        )
                    s6 = s6s[sl]
                    for p, d in enumerate([-PW2, 0, PW2, -1, 1]):
                        o = PW2 + 1 + d
                        nc.sync.dma_start(
                            out=s6[p:p + 1, 0:FL],
                            in_=scr_t[sl][o:o + FL].unsqueeze(0))
                    stg_t[sl] = stgpool.tile([COUT, FL], bf16, name=f"stg{sl}")

                if i >= 1:
                    pi_ = (i - 1) % 2
                    pimg = (i - 1) % BL
                    xqc = xq_t[pi_]
                    s6 = s6s[pi_]
                    stg = stg_t[pi_]
                    for wi in range(NW_C):
                        j0 = wi * 512
                        n = min(512, FL - j0)
                        ps = pmm.tile([COUT, 512], f32, name="psc")
                        first = True
                        for ky in range(3):
                            for kx in range(3):
                                t = ky * 3 + kx
                                off = j0 + PW2 * ky + kx
                                rhs = AP(xqc.tensor, xqc.offset + 2 * off,
                                         [list(xqc.ap[0]), [1, 2], [2, n]])
                                nc.tensor.matmul(
                                    ps[:, 0:n], wd[:, t], rhs,
                                    start=first, stop=False, perf_mode=DR)
                                first = False
                        nc.tensor.matmul(
                            ps[:, 0:n], sw, s6[:, j0:j0 + n],
                            start=False, stop=True)
                        if wi % 2 == 0:
                            nc.vector.tensor_scalar_add(
                                stg[:, j0:j0 + n], ps[:, 0:n], bias_sb)
                        else:
                            nc.scalar.activation(
                                out=stg[:, j0:j0 + n], in_=ps[:, 0:n],
                                func=AF.Identity, bias=bias_sb)
                    # replicate-pad border correction from s strips
                    scr = scr_t[pi_]
                    cs = corrpool.tile([1, 4 * W], bf16, name="csrc")
                    lsrc = AP(scr.tensor, scr.offset + PW2 + 1, [[PW2, H]])
                    nc.sync.dma_start(out=cs[:, 0:H], in_=lsrc.unsqueeze(0))
                    rsrc = AP(scr.tensor, scr.offset + PW2 + W, [[PW2, H]])
                    nc.sync.dma_start(out=cs[:, H:2 * H], in_=rsrc.unsqueeze(0))
                    nc.sync.dma_start(
                        out=cs[:, 2 * H:2 * H + W],
                        in_=scr[PW2 + 1:PW2 + 1 + W].unsqueeze(0))
                    nc.sync.dma_start(
                        out=cs[:, 2 * H + W:],
                        in_=scr[H * PW2 + 1:H * PW2 + 1 + W].unsqueeze(0))
                    ps_c = pbc.tile([COUT, 4 * W], f32, name="psb")
                    nc.tensor.matmul(ps_c, ones_row, cs, start=True, stop=True)
                    corr = corrpool.tile([COUT, 4 * W], bf16, name="corr")
                    nc.scalar.activation(out=corr, in_=ps_c, func=AF.Copy,
                                         scale=THETA)
                    vst = stg.rearrange("p (r c) -> p r c", c=PW2)
                    vco = corr.rearrange("p (s c) -> p s c", c=W)
                    nc.vector.tensor_tensor(
                        vst[:, :, 0:1], vst[:, :, 0:1],
                        vco[:, 0].unsqueeze(2), AL.subtract)
                    nc.vector.tensor_tensor(
                        vst[:, :, W - 1:W], vst[:, :, W - 1:W],
                        vco[:, 1].unsqueeze(2), AL.subtract)
                    nc.vector.tensor_tensor(
                        stg[:, 0:W], stg[:, 0:W], vco[:, 2], AL.subtract)
                    nc.vector.tensor_tensor(
                        stg[:, (H - 1) * PW2:(H - 1) * PW2 + W],
                        stg[:, (H - 1) * PW2:(H - 1) * PW2 + W],
                        vco[:, 3], AL.subtract)
                    nc.sync.dma_start(out=y_ext[pimg], in_=stg)
    _split_excess_waits(nc)
    return nc


def _prep_inputs_v2(x, Wm, b):
    import ml_dtypes

    xp = np.zeros((B, CIN, HP, PW2), np.float32)
    xp[:, :, 1:H + 1, 1:W + 1] = np.asarray(x, np.float32)
    xpf = xp.reshape(B, CIN, FLP)
    x8 = xpf.astype(ml_dtypes.float8_e4m3)
    r8 = ((xpf - x8.astype(np.float32)) * 8.0).astype(ml_dtypes.float8_e4m3)
    xq = np.zeros((B, CIN, XSZ, 2), ml_dtypes.float8_e4m3)
    xq[:, :, :FLP, 0] = x8
    xq[:, :, :FLP, 1] = r8
    xq = xq.reshape(B, CIN, 2 * XSZ)

    Wf = np.asarray(Wm, np.float32)
    wd = np.zeros((CIN, 9, 2, COUT), np.float32)
    for ky in range(3):
        for kx in range(3):
            t = ky * 3 + kx
            wt = Wf[:, :, ky, kx].T
            wd[:, t, 0] = wt
            wd[:, t, 1] = wt / 8.0
    wd8 = np.ascontiguousarray(
        wd.astype(ml_dtypes.float8_e4m3)).reshape(CIN, 18 * COUT)

    sq = np.zeros((CIN, 2, 16), np.float32)
    sq[:, 0, 0] = 1.0
    sq[:, 1, 0] = 0.125
    sq8 = sq.astype(ml_dtypes.float8_e4m3).reshape(CIN, 32)

    sw = np.zeros((5, COUT), np.float32)
    for p, cval in enumerate([-THETA, 4.0 * THETA, -THETA, -THETA, -THETA]):
        sw[p, :] = cval
    swb = sw.astype(ml_dtypes.bfloat16)
    bs = np.ascontiguousarray(np.asarray(b, np.float32).reshape(COUT, 1))

    feed = {
        "xq": xq,
        "wd": np.concatenate([wd8[None]] * N_CORES, 0).reshape(
            N_CORES * CIN, 18 * COUT),
        "sq": np.concatenate([sq8[None]] * N_CORES, 0).reshape(
            N_CORES * CIN, 32),
        "sw": np.concatenate([swb[None]] * N_CORES, 0).reshape(
            N_CORES * 5, COUT),
        "bias": np.concatenate([bs[None]] * N_CORES, 0).reshape(
            N_CORES * COUT, 1),
    }
    return feed


def _get_runner(compute, reps=1):
    """Compile once per process; returns (fn, in_names, out_names, shapes),
    sharded over the 8 cores."""
    key = (compute, reps)
    if key in _runner:
        return _runner[key]

    import jax
    import jax.numpy as jnp
    from jax.sharding import Mesh, PartitionSpec
    from jax.experimental.shard_map import shard_map
    import concourse.mybir as mybir
    from concourse import bass2jax

    if key not in _built:
        _built[key] = _build_v2(reps) if KERNEL_V2 else _build(compute, reps)
    nc = _built[key]

    bass2jax.install_neuronx_cc_hook()

    partition_name = (
        nc.partition_id_tensor.name if nc.partition_id_tensor else None
    )
    in_names, out_names, out_avals, zero_shapes = [], [], [], []
    for alloc in nc.m.functions[0].allocations:
        if not isinstance(alloc, mybir.MemoryLocationSet):
            continue
        name = alloc.memorylocations[0].name
        if alloc.kind == "ExternalInput":
            if name != partition_name:
                in_names.append(name)
        elif alloc.kind == "ExternalOutput":
            out_names.append(name)
            shape = tuple(alloc.tensor_shape)
            dtype = mybir.dt.np(alloc.dtype)
            out_avals.append(jax.core.ShapedArray(shape, dtype))
            zero_shapes.append((shape, dtype))
    n_params = len(in_names)
    all_in_names = in_names + out_names
    if partition_name is not None:
        all_in_names.append(partition_name)
    donate = tuple(range(n_params, n_params + len(out_names)))

    def _body(*args):
        operands = list(args)
        if partition_name is not None:
            operands.append(bass2jax.partition_id_tensor())
        outs = bass2jax._bass_exec_p.bind(
            *operands,
            out_avals=tuple(out_avals),
            in_names=tuple(all_in_names),
            out_names=tuple(out_names),
            lowering_input_output_aliases=(),
            sim_require_finite=True,
            sim_require_nnan=True,
            nc=nc,
        )
        return tuple(outs)

    devices = jax.devices()[:N_CORES]
    mesh = Mesh(np.asarray(devices), ("core",))
    nio = n_params + len(out_names)

    shape_by_name = {}
    for alloc in nc.m.functions[0].allocations:
        if not isinstance(alloc, mybir.MemoryLocationSet):
            continue
        name = alloc.memorylocations[0].name
        if alloc.kind in ("ExternalInput", "ExternalOutput"):
            shape_by_name[name] = (
                tuple(alloc.tensor_shape), mybir.dt.np(alloc.dtype)
            )
    sharding = jax.sharding.NamedSharding(mesh, PartitionSpec("core"))

    def _compile():
        jitted = jax.jit(
            shard_map(
                _body, mesh=mesh,
                in_specs=(PartitionSpec("core"),) * nio,
                out_specs=(PartitionSpec("core"),) * len(out_names),
                check_rep=False,
            ),
            donate_argnums=donate, keep_unused=True,
        )
        args = [
            jax.ShapeDtypeStruct(
                (N_CORES * shp[0], *shp[1:]), dt, sharding=sharding
            )
            for (shp, dt) in (
                shape_by_name[n] for n in in_names + out_names
            )
        ]
        return jitted.lower(*args).compile()

    # fast_dispatch_compile suppresses the per-call BassEffect so jit uses
    # the C++ fast dispatch path (saves ~6ms/call through the axon tunnel)
    try:
        sharded = bass2jax.fast_dispatch_compile(_compile)
    except Exception:
        sharded = jax.jit(
            shard_map(
                _body, mesh=mesh,
                in_specs=(PartitionSpec("core"),) * nio,
                out_specs=(PartitionSpec("core"),) * len(out_names),
                check_rep=False,
            ),
            donate_argnums=donate, keep_unused=True,
        )
    _runner[key] = (sharded, in_names, out_names, zero_shapes, sharding)
    return _runner[key]


def _prep_inputs(x, Wm, b, compute):
    if KERNEL_V2:
        return _prep_inputs_v2(x, Wm, b)
    import ml_dtypes

    cross = np.array([[0, 1, 0], [1, -4, 1], [0, 1, 0]], np.float32)
    Wf = np.asarray(Wm, np.float32) - THETA * cross[None, None]
    Wt = np.ascontiguousarray(Wf.transpose(1, 2, 3, 0)).reshape(CIN, 9 * COUT)
    npdt = np.float32 if compute == "f32r" else ml_dtypes.bfloat16
    xp = np.zeros((B, CIN, H, PW), npdt)
    xp[:, :, :, 1 : W + 1] = np.asarray(x)
    Wts = np.ascontiguousarray(Wt.astype(npdt, copy=False))
    bs = np.ascontiguousarray(np.asarray(b, np.float32).reshape(COUT, 1))
    ones = np.ones((CIN, 1), npdt)
    # global (concat over cores along axis 0) arrays for shard_map
    feed = {
        "x": xp,
        "Wt": np.concatenate([Wts[None]] * N_CORES, 0).reshape(N_CORES * CIN, 9 * COUT),
        "bias": np.concatenate([bs[None]] * N_CORES, 0).reshape(N_CORES * COUT, 1),
        "ones": np.concatenate([ones[None]] * N_CORES, 0).reshape(N_CORES * CIN, 1),
    }
    return feed


def _run(x, Wm, b, compute):
    import jax

    sharded, in_names, out_names, zero_shapes, sharding = _get_runner(compute)
    feed = _prep_inputs(x, Wm, b, compute)
    ins = [jax.device_put(feed[n], sharding) for n in in_names]
    zeros = [
        jax.device_put(np.zeros((N_CORES * s[0], *s[1:]), d), sharding)
        for (s, d) in zero_shapes
    ]
    outs = sharded(*ins, *zeros)
    y = np.asarray(outs[out_names.index("y")])
    if KERNEL_V2:
        y = y.reshape(B, COUT, H, PW2)[:, :, :, 0:W].astype(np.float32)
        return np.ascontiguousarray(y)
    return y.reshape(B, COUT, H, W).astype(np.float32)


def kernel(x, W, b):
    try:
        return _run(x, W, b, COMPUTE)
    except Exception:
        # one retry: transient device/terminal hiccups (e.g. a wedged core
        # from a previous session) usually clear on re-execution
        import time

        time.sleep(5.0)
        return _run(x, W, b, COMPUTE)



# revision 15
# speedup vs baseline: 1.4967x; 1.0027x over previous
"""Trainium2 Bass/Tile kernel: 3x3 conv (zero pad) + bias - theta * cross-stencil
(replicate pad) over NCHW f32, B=32, Cin=Cout=128, H=W=128, theta=0.7.

Math: the stencil term is a 3x3 conv with kernel [[0,1,0],[1,-4,1],[0,1,0]]
applied to sum_ci(x), identical for all (out,in) channel pairs.  For interior
pixels it folds into the conv weights:  W' = W - theta*cross.  The only
difference is at the 1-pixel image border where the stencil uses replicate
padding (out-of-bounds neighbor == edge value) while the conv uses zero
padding.  So:  out = conv_zp(x, W') + b - theta*corr, where corr adds
s=sum_ci(x) at each border pixel once per out-of-bounds neighbor (corners
twice).  corr is computed on-device from four border-strip channel sums
(ones-vector matmuls) broadcast across the 128 output channels.

Precision: weights/x in bf16 (PE rate equals f32r on TRN2: 1 col/cycle; the
measured stream rate here is ~0.51 ns/col, i.e. the PE sustains ~2 GHz under
full 8-core load), accumulation fp32 in PSUM, output staged and DMA'd as
bf16 and upcast on the host (rel err ~5e-3 vs the 2e-2 gate).  fp8 DoubleRow
was measured at only ~1.22x per tap on this toolchain (215.7 vs 264 ns per
512-col matmul) and loses more than that to the separate stencil machinery
it requires, so bf16-folded stays optimal (see BASS_KERNEL_V2 for the full
fp8 pipeline, kept for reference).

Sharding: data-parallel over batch, 4 images per core, 8 cores, SPMD.

Dispatch: the runner is AOT-compiled under bass2jax.fast_dispatch_compile;
one NEFF launch performs TIMING_REPS complete kernel applications, so the
~0.55 ms per-launch overhead amortizes to ~17 us/exec at reps=32.  Device
time is ~320-340 us/rep (PE-bound: 1152 conv matmuls x ~264 ns + corr).
"""

import os
import numpy as np

THETA = 0.7
N_CORES = 8
B, CIN, COUT, H, W = 32, 128, 128, 128, 128
BL = B // N_CORES          # images per core
PW = W + 2                 # horizontally padded row width in SBUF
ROWS_PER_BLK = 16          # output rows per SBUF staging tile / output DMA
CHUNK = 4                  # output rows per PSUM accumulation group (N=512)

COMPUTE = os.environ.get("BASS_CONV_DTYPE", "bf16")  # "f32r" | "bf16"
KERNEL_V2 = bool(os.environ.get("BASS_KERNEL_V2"))
# reps used by the steady-state timing harness: one NEFF launch performs
# TIMING_REPS complete kernel applications (full input DMA -> conv ->
# full output DMA each rep), amortizing the per-launch overhead the way a
# CUDA-graph-style batched timing loop would.  kernel() itself uses reps=1.
TIMING_REPS = int(os.environ.get("BASS_TIMING_REPS", "32"))

_built = {}
_runner = {}


def _patch_tile_drain():
    """This toolchain's walrus rejects instructions carrying more than one
    semaphore wait ('Too many sync wait commands' in setupSyncWait).  Tile's
    exit drain accumulates one wait per live semaphore on a single Drain, so
    re-emit those waits as a chain of single-wait NOPs in front of it."""
    import concourse.tile as tile
    import concourse.mybir as mybir
    from concourse.vector_clock import ScopedClock

    if getattr(tile.TileContext, "_drain_patched", False):
        return

    def _drain_and_barrier(self, tick_clock, wait_clock):
        nc = self.nc
        probe = nc.sync.nop(nofuse=True)
        wait_clock.add_sem_waits(
            probe.ins, ScopedClock({None: tick_clock.global_clock})
        )
        si = probe.ins.sync_info
        waits = list(si.on_wait) if si is not None and si.on_wait else []
        if len(waits) > 1:
            si.on_wait = waits[:1]
            for w in waits[1:]:
                nop = nc.sync.nop(nofuse=True)
                if nop.ins.sync_info is None:
                    nop.ins.sync_info = mybir.SyncInfo(on_wait=[w], on_update=[])
                else:
                    nop.ins.sync_info.on_wait = [w]
        nc.sync.drain()

        nc.all_engine_barrier()
        assert self.sems is not None
        popped = nc._tile_sem_poison_stack.pop()
        assert popped is self._sem_poison
        nc.clear_and_free_semaphores(list(self.sems.allocated().values()))
        nc.all_engine_barrier()

    tile.TileContext._drain_and_barrier = _drain_and_barrier
    tile.TileContext._drain_patched = True


def _split_excess_waits(nc, cap=1):
    """Hoist extra semaphore waits (walrus allows only `cap` per instruction
    on this toolchain) onto same-engine NOPs inserted just before the
    offending instruction."""
    import concourse.mybir as mybir

    n = 0
    for bb in nc.main_func.blocks:
        insts = bb.instructions
        out = []
        for inst in insts:
            si = inst.sync_info
            waits = list(si.on_wait) if si is not None and si.on_wait else []
            if len(waits) > cap:
                n += 1
                for i in range(0, len(waits) - cap, cap):
                    chunk = waits[i : i + cap]
                    nop = mybir.InstNoOp(
                        name=nc.get_next_instruction_name(),
                        sync_info=mybir.SyncInfo(on_wait=list(chunk), on_update=[]),
                        engine=inst.engine,
                        bass_nofuse=True,
                    )
                    nc.register_instruction(nop)
                    out.append(nop)
                si.on_wait = waits[len(waits) - cap :]
            out.append(inst)
        insts[:] = out
    return n


def _build(compute, reps=1):
    import concourse.bass as bass
    import concourse.mybir as mybir
    import concourse.tile as tile

    _patch_tile_drain()
    cdt = {"f32r": mybir.dt.float32r, "bf16": mybir.dt.bfloat16}[compute]
    f32 = mybir.dt.float32
    AF = mybir.ActivationFunctionType

    nc = bass.Bass()
    # x arrives pre-padded on the host: two zero columns per row (PW=W+2),
    # so the image DMA is one contiguous copy and zero-padding needs no
    # on-device memsets (f32r memset fails the walrus ISA check).
    x_ext = nc.declare_dram_parameter("x", [BL, CIN, H, PW], cdt, isOutput=False)
    wt_ext = nc.declare_dram_parameter("Wt", [CIN, 9 * COUT], cdt, isOutput=False)
    b_ext = nc.declare_dram_parameter("bias", [COUT, 1], f32, isOutput=False)
    ones_ext = nc.declare_dram_parameter("ones", [CIN, 1], cdt, isOutput=False)
    y_ext = nc.declare_dram_parameter("y", [BL, COUT, H, W], mybir.dt.bfloat16,
                                      isOutput=True)

    NBLK = H // ROWS_PER_BLK
    NCH = ROWS_PER_BLK // CHUNK

    with tile.TileContext(nc) as tc:
        with (
            tc.tile_pool(name="singles", bufs=1) as singles,
            tc.tile_pool(name="xin", bufs=2) as xpool,
            tc.tile_pool(name="outs", bufs=3) as outpool,
            tc.tile_pool(name="corr", bufs=1) as corrpool,
            tc.tile_pool(name="pmm", bufs=4, space="PSUM") as pmm,
            tc.tile_pool(name="psmall", bufs=2, space="PSUM") as psmall,
            tc.tile_pool(name="pbcast", bufs=2, space="PSUM") as pbcast,
        ):
            w_sb = singles.tile([CIN, 9 * COUT], cdt)
            nc.sync.dma_start(out=w_sb, in_=wt_ext[:])
            bias_sb = singles.tile([COUT, 1], f32)
            nc.sync.dma_start(out=bias_sb, in_=b_ext[:])
            ones_col = singles.tile([CIN, 1], cdt)
            nc.sync.dma_start(out=ones_col, in_=ones_ext[:])
            ones_row = singles.tile([1, COUT], f32)
            nc.vector.memset(ones_row, 1.0)

            for img_rep in range(BL * reps):
                img = img_rep % BL
                x_t = xpool.tile([CIN, H, PW], cdt)
                nc.sync.dma_start(out=x_t, in_=x_ext[img])

                # Channel sums of the four border strips -> one PSUM bank:
                # [0:H) left col, [H:2H) right col, [2H:2H+W) top, [2H+W:) bottom
                ps_s = psmall.tile([1, 2 * H + 2 * W], f32)
                nc.tensor.matmul(
                    ps_s[:, 0:H], ones_col, x_t[:, :, 1:2], start=True, stop=False
                )
                nc.tensor.matmul(
                    ps_s[:, H : 2 * H], ones_col, x_t[:, :, W : W + 1],
                    start=False, stop=False,
                )
                nc.tensor.matmul(
                    ps_s[:, 2 * H : 2 * H + W], ones_col, x_t[:, 0:1, 1 : W + 1],
                    start=False, stop=False,
                )
                nc.tensor.matmul(
                    ps_s[:, 2 * H + W :], ones_col, x_t[:, H - 1 : H, 1 : W + 1],
                    start=False, stop=True,
                )
                s_sb = corrpool.tile([1, 2 * H + 2 * W], f32)
                nc.scalar.activation(out=s_sb, in_=ps_s, func=AF.Copy)
                # broadcast across the 128 out-channel partitions, scaled by theta
                ps_c = pbcast.tile([COUT, 2 * H + 2 * W], f32)
                nc.tensor.matmul(ps_c, ones_row, s_sb, start=True, stop=True)
                corr_sb = corrpool.tile([COUT, 2 * H + 2 * W],
                                        mybir.dt.bfloat16)
                nc.scalar.activation(out=corr_sb, in_=ps_c, func=AF.Copy, scale=THETA)

                for blk in range(NBLK):
                    out_sb = outpool.tile([COUT, ROWS_PER_BLK * W],
                                          mybir.dt.bfloat16)
                    for q in range(NCH):
                        y0 = blk * ROWS_PER_BLK + q * CHUNK
                        ps = pmm.tile([COUT, CHUNK * W], f32)
                        first = True
                        for ky in range(3):
                            r0 = y0 + ky - 1
                            rows, out_off = CHUNK, 0
                            if r0 < 0:
                                r0, rows, out_off = 0, CHUNK - 1, W
                            elif r0 + CHUNK > H:
                                rows = H - r0
                            for kx in range(3):
                                t = 3 * ky + kx
                                nc.tensor.matmul(
                                    ps[:, out_off : out_off + rows * W],
                                    w_sb[:, t * COUT : (t + 1) * COUT],
                                    x_t[:, r0 : r0 + rows, kx : kx + W],
                                    start=first, stop=(t == 8),
                                )
                                first = False
                        nc.vector.tensor_scalar_add(
                            out_sb[:, q * CHUNK * W : (q + 1) * CHUNK * W],
                            ps, bias_sb,
                        )
                    # border corrections (replicate-pad delta)
                    v = out_sb.rearrange("p (r c) -> p r c", c=W)
                    r0 = blk * ROWS_PER_BLK
                    r1 = r0 + ROWS_PER_BLK
                    cl = corr_sb[:, r0:r1].rearrange("p (r c) -> p r c", c=1)
                    cr = corr_sb[:, H + r0 : H + r1].rearrange("p (r c) -> p r c", c=1)
                    nc.vector.tensor_sub(v[:, :, 0:1], v[:, :, 0:1], cl)
                    nc.vector.tensor_sub(v[:, :, W - 1 : W], v[:, :, W - 1 : W], cr)
                    if blk == 0:
                        nc.vector.tensor_sub(
                            out_sb[:, 0:W], out_sb[:, 0:W],
                            corr_sb[:, 2 * H : 2 * H + W],
                        )
                    if blk == NBLK - 1:
                        last = (ROWS_PER_BLK - 1) * W
                        nc.vector.tensor_sub(
                            out_sb[:, last : last + W], out_sb[:, last : last + W],
                            corr_sb[:, 2 * H + W :],
                        )
                    nc.sync.dma_start(
                        out=y_ext[img, :, r0:r1, :],
                        in_=out_sb.rearrange("p (r c) -> p r c", c=W),
                    )
    _split_excess_waits(nc)
    return nc


# ---------------------------------------------------------------------------
# v2: fp8 DoubleRow conv over flat padded windows + bf16 stencil tap.
#
# Layouts (per core, all spatial data in "padded flat" form):
#   padded image: HP x PW = 130 x 130 (zero pad ring), flat length FLP=16900
#   output flat:  FL = H*PW = 16640 (row r at flat r*PW, cols 128/129 garbage)
#   tap (ky,kx) of output j reads x8p_flat[j + 130*ky + kx]
# Conv: 6 fp8 DoubleRow pair-matmuls per 512-wide window (w_ky1 split in half
# across the two pairs; halving is exact in fp8), accumulating with a 7th
# bf16 matmul of contraction 6 that adds  bias + theta*(4c - n-s-e-w)  from
# an s-plane stack (s = channel sum of x, bf16), computed one image ahead:
# 52 col-group-tiled ones-matmuls over xb -> PSUM (4 windows of 325 share a
# bank via tile_position) -> ACT evac -> DRAM scratch -> 5 shifted contiguous
# DMA reads back as planes 1-5 (plane 0 = ones, memset once, carries bias).
# Replicate-pad delta handled like v1: border strips of s broadcast via a
# ones-row matmul and subtracted from the staging borders.
# ---------------------------------------------------------------------------

HP = H + 2
PW2 = W + 2
FLP = HP * PW2           # 16900
FL = H * PW2             # 16640
XSZ = FLP + 2            # +2 sentinel zeros for the last conv window reads
WSZ_S = 325              # s-window size: 52 windows, 13 evac groups x 4
NW_S = FLP // WSZ_S      # 52
NW_C = (FL + 511) // 512  # 33 conv windows


def _build_v2(reps=1):
    import concourse.bass as bass
    import concourse.mybir as mybir
    import concourse.tile as tile
    from concourse.ap import AP

    _patch_tile_drain()
    f32 = mybir.dt.float32
    bf16 = mybir.dt.bfloat16
    fp8 = mybir.dt.float8e4
    AF = mybir.ActivationFunctionType
    AL = mybir.AluOpType
    DR = mybir.MatmulPerfMode.DoubleRow

    nc = bass.Bass()
    xq_ext = nc.declare_dram_parameter("xq", [BL, CIN, 2 * XSZ], fp8,
                                       isOutput=False)
    wd_ext = nc.declare_dram_parameter("wd", [CIN, 18 * COUT], fp8,
                                       isOutput=False)
    sq_ext = nc.declare_dram_parameter("sq", [CIN, 2 * 16], fp8, isOutput=False)
    sw_ext = nc.declare_dram_parameter("sw", [5, COUT], bf16, isOutput=False)
    b_ext = nc.declare_dram_parameter("bias", [COUT, 1], f32, isOutput=False)
    y_ext = nc.declare_dram_parameter("y", [BL, COUT, FL], bf16, isOutput=True)

    NIT = BL * reps

    with tile.TileContext(nc) as tc:
        with (
            tc.tile_pool(name="singles", bufs=1) as singles,
            tc.tile_pool(name="xqp", bufs=1) as xqpool,
            tc.tile_pool(name="stg", bufs=1) as stgpool,
            tc.tile_pool(name="sbfp", bufs=1) as sbfpool,
            tc.tile_pool(name="corrp", bufs=1) as corrpool,
            tc.tile_pool(name="pmm", bufs=4, space="PSUM") as pmm,
            tc.tile_pool(name="psm", bufs=2, space="PSUM") as psm,
            tc.tile_pool(name="pbc", bufs=1, space="PSUM") as pbc,
            tc.tile_pool(name="scrp", bufs=1, space="DRAM") as scrpool,
        ):
            wd = singles.tile([CIN, 9, 2, COUT], fp8)
            nc.sync.dma_start(
                out=wd,
                in_=wd_ext[:].rearrange("p (t k o) -> p t k o", k=2, o=COUT))
            sq = singles.tile([CIN, 2, 16], fp8)
            nc.sync.dma_start(
                out=sq, in_=sq_ext[:].rearrange("p (k o) -> p k o", k=2))
            sw = singles.tile([5, COUT], bf16)
            nc.sync.dma_start(out=sw, in_=sw_ext[:])
            bias_sb = singles.tile([COUT, 1], f32)
            nc.sync.dma_start(out=bias_sb, in_=b_ext[:])
            ones_row = singles.tile([1, COUT], bf16)
            nc.vector.memset(ones_row, 1.0)
            s6a = singles.tile([5, FL], bf16)
            s6b = singles.tile([5, FL], bf16)
            s6s = [s6a, s6b]

            xq_t = [None, None]
            stg_t = [None, None]
            scr_t = [None, None]

            for i in range(NIT + 1):
                if i <= NIT - 1:
                    img = i % BL
                    sl = i % 2
                    xq_t[sl] = xqpool.tile([CIN, 2 * XSZ], fp8,
                                           name=f"xqt{sl}")
                    nc.sync.dma_start(out=xq_t[sl], in_=xq_ext[img])
                    xq = xq_t[sl]

                    scr_t[sl] = scrpool.tile([FLP + 1], bf16, name=f"scr{sl}")
                    for batch in range(13):
                        s_bf = sbfpool.tile([16, 4 * WSZ_S], bf16, name="sbf")
                        for k in range(4):
                            wnd = 4 * batch + k
                            j0 = WSZ_S * wnd
                            ps_s = psm.tile([16, WSZ_S], f32, name="pss")
                            rhs = AP(xq.tensor, xq.offset + 2 * j0,
                                     [list(xq.ap[0]), [1, 2], [2, WSZ_S]])
                            nc.tensor.matmul(ps_s, sq, rhs, start=True,
                                             stop=True, perf_mode=DR)
                            dstc = s_bf[:, k * WSZ_S:(k + 1) * WSZ_S]
                            if k % 2 == 0:
                                nc.vector.tensor_copy(dstc, ps_s)
                            else:
                                nc.scalar.activation(out=dstc, in_=ps_s,
                                                     func=AF.Copy)
                        nc.sync.dma_start(
                            out=scr_t[sl][batch * 4 * WSZ_S:
                                          (batch + 1) * 4 * WSZ_S].unsqueeze(0),
                            in_=s_bf[0:1],
                        )
                    s6 = s6s[sl]
                    for p, d in enumerate([-PW2, 0, PW2, -1, 1]):
                        o = PW2 + 1 + d
                        nc.sync.dma_start(
                            out=s6[p:p + 1, 0:FL],
                            in_=scr_t[sl][o:o + FL].unsqueeze(0))
                    stg_t[sl] = stgpool.tile([COUT, FL], bf16, name=f"stg{sl}")

                if i >= 1:
                    pi_ = (i - 1) % 2
                    pimg = (i - 1) % BL
                    xqc = xq_t[pi_]
                    s6 = s6s[pi_]
                    stg = stg_t[pi_]
                    for wi in range(NW_C):
                        j0 = wi * 512
                        n = min(512, FL - j0)
                        ps = pmm.tile([COUT, 512], f32, name="psc")
                        first = True
                        for ky in range(3):
                            for kx in range(3):
                                t = ky * 3 + kx
                                off = j0 + PW2 * ky + kx
                                rhs = AP(xqc.tensor, xqc.offset + 2 * off,
                                         [list(xqc.ap[0]), [1, 2], [2, n]])
                                nc.tensor.matmul(
                                    ps[:, 0:n], wd[:, t], rhs,
                                    start=first, stop=False, perf_mode=DR)
                                first = False
                        nc.tensor.matmul(
                            ps[:, 0:n], sw, s6[:, j0:j0 + n],
                            start=False, stop=True)
                        if wi % 2 == 0:
                            nc.vector.tensor_scalar_add(
                                stg[:, j0:j0 + n], ps[:, 0:n], bias_sb)
                        else:
                            nc.scalar.activation(
                                out=stg[:, j0:j0 + n], in_=ps[:, 0:n],
                                func=AF.Identity, bias=bias_sb)
                    # replicate-pad border correction from s strips
                    scr = scr_t[pi_]
                    cs = corrpool.tile([1, 4 * W], bf16, name="csrc")
                    lsrc = AP(scr.tensor, scr.offset + PW2 + 1, [[PW2, H]])
                    nc.sync.dma_start(out=cs[:, 0:H], in_=lsrc.unsqueeze(0))
                    rsrc = AP(scr.tensor, scr.offset + PW2 + W, [[PW2, H]])
                    nc.sync.dma_start(out=cs[:, H:2 * H], in_=rsrc.unsqueeze(0))
                    nc.sync.dma_start(
                        out=cs[:, 2 * H:2 * H + W],
                        in_=scr[PW2 + 1:PW2 + 1 + W].unsqueeze(0))
                    nc.sync.dma_start(
                        out=cs[:, 2 * H + W:],
                        in_=scr[H * PW2 + 1:H * PW2 + 1 + W].unsqueeze(0))
                    ps_c = pbc.tile([COUT, 4 * W], f32, name="psb")
                    nc.tensor.matmul(ps_c, ones_row, cs, start=True, stop=True)
                    corr = corrpool.tile([COUT, 4 * W], bf16, name="corr")
                    nc.scalar.activation(out=corr, in_=ps_c, func=AF.Copy,
                                         scale=THETA)
                    vst = stg.rearrange("p (r c) -> p r c", c=PW2)
                    vco = corr.rearrange("p (s c) -> p s c", c=W)
                    nc.vector.tensor_tensor(
                        vst[:, :, 0:1], vst[:, :, 0:1],
                        vco[:, 0].unsqueeze(2), AL.subtract)
                    nc.vector.tensor_tensor(
                        vst[:, :, W - 1:W], vst[:, :, W - 1:W],
                        vco[:, 1].unsqueeze(2), AL.subtract)
                    nc.vector.tensor_tensor(
                        stg[:, 0:W], stg[:, 0:W], vco[:, 2], AL.subtract)
                    nc.vector.tensor_tensor(
                        stg[:, (H - 1) * PW2:(H - 1) * PW2 + W],
                        stg[:, (H - 1) * PW2:(H - 1) * PW2 + W],
                        vco[:, 3], AL.subtract)
                    nc.sync.dma_start(out=y_ext[pimg], in_=stg)
    _split_excess_waits(nc)
    return nc


def _prep_inputs_v2(x, Wm, b):
    import ml_dtypes

    xp = np.zeros((B, CIN, HP, PW2), np.float32)
    xp[:, :, 1:H + 1, 1:W + 1] = np.asarray(x, np.float32)
    xpf = xp.reshape(B, CIN, FLP)
    x8 = xpf.astype(ml_dtypes.float8_e4m3)
    r8 = ((xpf - x8.astype(np.float32)) * 8.0).astype(ml_dtypes.float8_e4m3)
    xq = np.zeros((B, CIN, XSZ, 2), ml_dtypes.float8_e4m3)
    xq[:, :, :FLP, 0] = x8
    xq[:, :, :FLP, 1] = r8
    xq = xq.reshape(B, CIN, 2 * XSZ)

    Wf = np.asarray(Wm, np.float32)
    wd = np.zeros((CIN, 9, 2, COUT), np.float32)
    for ky in range(3):
        for kx in range(3):
            t = ky * 3 + kx
            wt = Wf[:, :, ky, kx].T
            wd[:, t, 0] = wt
            wd[:, t, 1] = wt / 8.0
    wd8 = np.ascontiguousarray(
        wd.astype(ml_dtypes.float8_e4m3)).reshape(CIN, 18 * COUT)

    sq = np.zeros((CIN, 2, 16), np.float32)
    sq[:, 0, 0] = 1.0
    sq[:, 1, 0] = 0.125
    sq8 = sq.astype(ml_dtypes.float8_e4m3).reshape(CIN, 32)

    sw = np.zeros((5, COUT), np.float32)
    for p, cval in enumerate([-THETA, 4.0 * THETA, -THETA, -THETA, -THETA]):
        sw[p, :] = cval
    swb = sw.astype(ml_dtypes.bfloat16)
    bs = np.ascontiguousarray(np.asarray(b, np.float32).reshape(COUT, 1))

    feed = {
        "xq": xq,
        "wd": np.concatenate([wd8[None]] * N_CORES, 0).reshape(
            N_CORES * CIN, 18 * COUT),
        "sq": np.concatenate([sq8[None]] * N_CORES, 0).reshape(
            N_CORES * CIN, 32),
        "sw": np.concatenate([swb[None]] * N_CORES, 0).reshape(
            N_CORES * 5, COUT),
        "bias": np.concatenate([bs[None]] * N_CORES, 0).reshape(
            N_CORES * COUT, 1),
    }
    return feed


def _get_runner(compute, reps=1):
    """Compile once per process; returns (fn, in_names, out_names, shapes),
    sharded over the 8 cores."""
    key = (compute, reps)
    if key in _runner:
        return _runner[key]

    import jax
    import jax.numpy as jnp
    from jax.sharding import Mesh, PartitionSpec
    from jax.experimental.shard_map import shard_map
    import concourse.mybir as mybir
    from concourse import bass2jax

    if key not in _built:
        _built[key] = _build_v2(reps) if KERNEL_V2 else _build(compute, reps)
    nc = _built[key]

    bass2jax.install_neuronx_cc_hook()

    partition_name = (
        nc.partition_id_tensor.name if nc.partition_id_tensor else None
    )
    in_names, out_names, out_avals, zero_shapes = [], [], [], []
    for alloc in nc.m.functions[0].allocations:
        if not isinstance(alloc, mybir.MemoryLocationSet):
            continue
        name = alloc.memorylocations[0].name
        if alloc.kind == "ExternalInput":
            if name != partition_name:
                in_names.append(name)
        elif alloc.kind == "ExternalOutput":
            out_names.append(name)
            shape = tuple(alloc.tensor_shape)
            dtype = mybir.dt.np(alloc.dtype)
            out_avals.append(jax.core.ShapedArray(shape, dtype))
            zero_shapes.append((shape, dtype))
    n_params = len(in_names)
    all_in_names = in_names + out_names
    if partition_name is not None:
        all_in_names.append(partition_name)
    donate = tuple(range(n_params, n_params + len(out_names)))

    def _body(*args):
        operands = list(args)
        if partition_name is not None:
            operands.append(bass2jax.partition_id_tensor())
        outs = bass2jax._bass_exec_p.bind(
            *operands,
            out_avals=tuple(out_avals),
            in_names=tuple(all_in_names),
            out_names=tuple(out_names),
            lowering_input_output_aliases=(),
            sim_require_finite=True,
            sim_require_nnan=True,
            nc=nc,
        )
        return tuple(outs)

    devices = jax.devices()[:N_CORES]
    mesh = Mesh(np.asarray(devices), ("core",))
    nio = n_params + len(out_names)

    shape_by_name = {}
    for alloc in nc.m.functions[0].allocations:
        if not isinstance(alloc, mybir.MemoryLocationSet):
            continue
        name = alloc.memorylocations[0].name
        if alloc.kind in ("ExternalInput", "ExternalOutput"):
            shape_by_name[name] = (
                tuple(alloc.tensor_shape), mybir.dt.np(alloc.dtype)
            )
    sharding = jax.sharding.NamedSharding(mesh, PartitionSpec("core"))

    def _compile():
        jitted = jax.jit(
            shard_map(
                _body, mesh=mesh,
                in_specs=(PartitionSpec("core"),) * nio,
                out_specs=(PartitionSpec("core"),) * len(out_names),
                check_rep=False,
            ),
            donate_argnums=donate, keep_unused=True,
        )
        args = [
            jax.ShapeDtypeStruct(
                (N_CORES * shp[0], *shp[1:]), dt, sharding=sharding
            )
            for (shp, dt) in (
                shape_by_name[n] for n in in_names + out_names
            )
        ]
        return jitted.lower(*args).compile()

    # fast_dispatch_compile suppresses the per-call BassEffect so jit uses
    # the C++ fast dispatch path (saves ~6ms/call through the axon tunnel)
    try:
        sharded = bass2jax.fast_dispatch_compile(_compile)
    except Exception:
        sharded = jax.jit(
            shard_map(
                _body, mesh=mesh,
                in_specs=(PartitionSpec("core"),) * nio,
                out_specs=(PartitionSpec("core"),) * len(out_names),
                check_rep=False,
            ),
            donate_argnums=donate, keep_unused=True,
        )
    _runner[key] = (sharded, in_names, out_names, zero_shapes, sharding)
    return _runner[key]


def _prep_inputs(x, Wm, b, compute):
    if KERNEL_V2:
        return _prep_inputs_v2(x, Wm, b)
    import ml_dtypes

    cross = np.array([[0, 1, 0], [1, -4, 1], [0, 1, 0]], np.float32)
    Wf = np.asarray(Wm, np.float32) - THETA * cross[None, None]
    Wt = np.ascontiguousarray(Wf.transpose(1, 2, 3, 0)).reshape(CIN, 9 * COUT)
    npdt = np.float32 if compute == "f32r" else ml_dtypes.bfloat16
    xp = np.zeros((B, CIN, H, PW), npdt)
    xp[:, :, :, 1 : W + 1] = np.asarray(x)
    Wts = np.ascontiguousarray(Wt.astype(npdt, copy=False))
    bs = np.ascontiguousarray(np.asarray(b, np.float32).reshape(COUT, 1))
    ones = np.ones((CIN, 1), npdt)
    # global (concat over cores along axis 0) arrays for shard_map
    feed = {
        "x": xp,
        "Wt": np.concatenate([Wts[None]] * N_CORES, 0).reshape(N_CORES * CIN, 9 * COUT),
        "bias": np.concatenate([bs[None]] * N_CORES, 0).reshape(N_CORES * COUT, 1),
        "ones": np.concatenate([ones[None]] * N_CORES, 0).reshape(N_CORES * CIN, 1),
    }
    return feed


def _run(x, Wm, b, compute):
    import jax

    sharded, in_names, out_names, zero_shapes, sharding = _get_runner(compute)
    feed = _prep_inputs(x, Wm, b, compute)
    ins = [jax.device_put(feed[n], sharding) for n in in_names]
    zeros = [
        jax.device_put(np.zeros((N_CORES * s[0], *s[1:]), d), sharding)
        for (s, d) in zero_shapes
    ]
    outs = sharded(*ins, *zeros)
    y = np.asarray(outs[out_names.index("y")])
    if KERNEL_V2:
        y = y.reshape(B, COUT, H, PW2)[:, :, :, 0:W].astype(np.float32)
        return np.ascontiguousarray(y)
    return y.reshape(B, COUT, H, W).astype(np.float32)


def kernel(x, W, b):
    try:
        return _run(x, W, b, COMPUTE)
    except Exception:
        # one retry: transient device/terminal hiccups (e.g. a wedged core
        # from a previous session) usually clear on re-execution
        import time

        time.sleep(5.0)
        return _run(x, W, b, COMPUTE)

